# revision 19
# baseline (speedup 1.0000x reference)
"""Self-contained Trainium2 (Bass) kernel for the 3-layer GCN
nn_FeaturePropagationModule problem: 100K nodes, 1.6M edges,
dims 512->64->128->40, log_softmax output, 8 NeuronCores.

Strategy (sharding_hint: shard nodes + edges by destination, replicate
weights): nodes are permuted into 8 shards x 98 blocks x 128 dsts
(degree-balanced); per layer each core computes its shard's y table
(y = scaled h @ W), AllGathers the bf16 table, then aggregates its own
dst blocks via dma_gather of 256B source rows + one-hot scatter-add
matmuls accumulated in PSUM.

v2 (vs the first working version): gathers are grouped 7 dst-blocks at
a time (one gather per (block-group, src-bucket) instead of per
(block, bucket)) cutting SWDGE descriptor-generation fixed overhead
(994ns/instr) ~7x; the one-hot S is built in [edge, dst, chunk] layout
from all-bf16 packed operands so the DVE 2x_1p mode applies; L1/L2
aggregation runs transposed (paT = msg^T @ S) so the epilogue is a
single Relu straight into the next layer's lhsT table (no transpose /
copy), with the symmetric-norm scale folded into the next y-phase's
PSUM->SBUF copy (relu(d^2*agg) = d*relu(d*agg)); edge indices and
dst-column tables are SBUF-resident, loaded once; x is staged
block-major so L1 streams 1KB-contiguous tiles.
"""
import numpy as np

import concourse.bacc as bacc
import concourse.mybir as mybir
from concourse.bass_utils import run_bass_kernel_spmd
from concourse.masks import make_identity
from concourse.tile import TileContext

FP = mybir.dt.float32
BF = mybir.dt.bfloat16
I16 = mybir.dt.int16
TW = 128  # gather-table width (bf16 -> 256B rows)
KCAP = 8  # max chunks per dma_gather (1024-idx hard limit, probed)
NCORES = 8
N_NODES = 100000
NB_BLOCKS = 98
GRP = 7  # dst blocks per gather group


# ---------------- host-side preprocessing ----------------


def _balanced_assignment(src, dst, N, NB):
    """Two-stage node->position assignment minimizing chunk padding.

    Stage 1: nodes -> 4 core-pairs (= src buckets) by out-degree snake.
    Stage 2: per pair, round-based packing of nodes into the pair's
    2*NB (core, block) bins of 128, balancing each bin's 4-vector of
    per-bucket in-degrees (self-loops excluded: they never enter the
    gather path).
    """
    SHARD = NB * 128
    PADN = NCORES * SHARD

    outdeg = np.bincount(src, minlength=N) + 1.0
    order = np.argsort(-outdeg, kind="stable")
    pair_of_node = np.empty(N, dtype=np.int64)
    for i, n_ in enumerate(order):
        r, c = divmod(i, 4)
        pair_of_node[n_] = c if r % 2 == 0 else 3 - c

    indegq = np.zeros((N, 4), dtype=np.int32)
    np.add.at(indegq, (dst, pair_of_node[src]), 1)

    caps = np.full((NB, 4), 590.0)
    NBINP = 2 * NB
    cap_bins = np.vstack([caps, caps])  # [2*NB, 4]

    perm_of_node = np.full(N, -1, dtype=np.int64)
    node_of_perm = np.full(PADN, -1, dtype=np.int64)
    for p in range(4):
        nodes = np.where(pair_of_node == p)[0]
        w = indegq[nodes].astype(np.float64)
        o = np.argsort(-w.sum(axis=1), kind="stable")
        nodes, w = nodes[o], w[o]
        npair = len(nodes)
        loads = np.zeros((NBINP, 4))
        fill = np.zeros(NBINP, dtype=np.int64)
        assign = np.empty(npair, dtype=np.int64)
        pos = 0
        for r in range(128):
            take = min(NBINP, npair - pos)
            if take <= 0:
                break
            for i in range(pos, pos + take):
                newload = loads + w[i]
                relfill = (newload / cap_bins).max(axis=1)
                infeas = (fill != r) | (newload > cap_bins).any(axis=1)
                score = np.where(infeas, np.inf, relfill)
                bi = int(np.argmin(score))
                if not np.isfinite(score[bi]):
                    over = np.where(fill != r, np.inf,
                                    (newload - cap_bins).max(axis=1))
                    bi = int(np.argmin(over))
                assign[i] = bi
                loads[bi] += w[i]
                fill[bi] += 1
            pos += take
        cnt = np.zeros(NBINP, dtype=np.int64)
        for i, n_ in enumerate(nodes):
            bi = assign[i]
            core = 2 * p + bi // NB
            blk = bi % NB
            ppos = core * SHARD + blk * 128 + cnt[bi]
            cnt[bi] += 1
            perm_of_node[n_] = ppos
            node_of_perm[ppos] = n_
    return perm_of_node, node_of_perm


def preprocess(edge_index: np.ndarray, N: int, NB: int):
    G = GRP
    NG = NB // G
    assert NB % G == 0
    SHARD = NB * 128
    PADN = NCORES * SHARD
    BUCKET = PADN // 4
    assert BUCKET < 32768 and N <= PADN
    src = edge_index[0].astype(np.int64)
    dst = edge_index[1].astype(np.int64)

    deg = np.bincount(dst, minlength=N).astype(np.float64) + 1.0
    dis = (1.0 / np.sqrt(deg)).astype(np.float32)

    perm_of_node, node_of_perm = _balanced_assignment(src, dst, N, NB)

    # permuted edge list; self-loops are NOT gathered (their contribution
    # is added locally from ybuf via a diagonal-matmul PSUM seed)
    psrc = perm_of_node[src]
    pdst = perm_of_node[dst]

    core_of = pdst // SHARD
    blk_of = (pdst % SHARD) // 128
    dcol_of = pdst % 128
    bucket_of = psrc // BUCKET

    counts = np.zeros((NCORES, NB, 4), dtype=np.int64)
    np.add.at(counts, (core_of, blk_of, bucket_of), 1)
    kbq = np.ceil(counts / 128).astype(np.int64).max(axis=0)  # [NB, 4]

    # chunk layout: (group, bucket, block)-major
    calls = []  # (g, q, ch0, runs=((b, k), ...))
    ch = 0
    ch0_of_bq = np.full((NB, 4), -1, dtype=np.int64)
    for g in range(NG):
        for q in range(4):
            runs = []
            ch0 = ch
            for b in range(g * G, (g + 1) * G):
                k = int(kbq[b, q])
                if k == 0:
                    continue
                ch0_of_bq[b, q] = ch
                runs.append((b, k))
                ch += k
            if runs:
                calls.append((g, q, ch0, tuple(runs)))
    NCH = ch

    # per-chunk (is_first, is_last) for PSUM accumulation groups; the
    # group is STARTED by the per-block self-term matmul, so is_first is
    # always False. selfstop[b]: block has no gathered chunks at all.
    flags = [[False, False] for _ in range(NCH)]
    selfstop = [False] * NB
    blk_of_ch = np.zeros(NCH, dtype=np.int64)
    for b in range(NB):
        chs = []
        for q in range(4):
            k = int(kbq[b, q])
            if k:
                c0 = int(ch0_of_bq[b, q])
                chs.extend(range(c0, c0 + k))
        if not chs:
            selfstop[b] = True
            continue
        flags[chs[-1]][1] = True
        for c in chs:
            blk_of_ch[c] = b

    idx16 = np.zeros((NCORES, 128, NCH * 8), dtype=np.int16)
    dcol = np.full((NCORES, 128, NCH), 255, dtype=np.float32)
    for c in range(NCORES):
        m = core_of == c
        eb, eq, ed, es = blk_of[m], bucket_of[m], dcol_of[m], psrc[m]
        o = np.lexsort((es, eb, eq))
        eb, eq, ed, es = eb[o], eq[o], ed[o], es[o]
        # edges now sorted by (bucket, block, src); offsets per (q, b)
        key = eq * NB + eb
        orderqb = np.argsort(key, kind="stable")
        eb, eq, ed, es = eb[orderqb], eq[orderqb], ed[orderqb], es[orderqb]
        cnts = np.bincount(key[orderqb], minlength=4 * NB)
        starts = np.concatenate([[0], np.cumsum(cnts)])
        for q in range(4):
            for b in range(NB):
                cnt = int(cnts[q * NB + b])
                k = int(kbq[b, q])
                if k == 0:
                    assert cnt == 0
                    continue
                assert cnt <= k * 128
                p0 = int(starts[q * NB + b])
                loc = es[p0:p0 + cnt] - q * BUCKET
                dcs = ed[p0:p0 + cnt]
                c0 = int(ch0_of_bq[b, q])
                flat_idx = np.zeros(k * 128, dtype=np.int16)
                flat_idx[:cnt] = loc.astype(np.int16)
                flat_dc = np.full(k * 128, 255, dtype=np.float32)
                flat_dc[:cnt] = dcs.astype(np.float32)
                cols = flat_idx.reshape(k * 8, 16).T
                for gg in range(8):
                    idx16[c, gg * 16:(gg + 1) * 16,
                          c0 * 8:(c0 + k) * 8] = cols
                dcol[c, :, c0:c0 + k] = flat_dc.reshape(k, 128).T

    dis_pad = np.zeros(PADN, dtype=np.float32)
    real = node_of_perm >= 0
    dis_pad[real] = dis[node_of_perm[real]]
    dis_cb = dis_pad.reshape(NCORES, NB, 128).transpose(0, 2, 1).copy()

    Kmax = max(sum(k for _, k in runs) for (_, _, _, runs) in calls)

    return dict(
        perm_of_node=perm_of_node, node_of_perm=node_of_perm, PADN=PADN,
        SHARD=SHARD, NB=NB, BUCKET=BUCKET, NCH=NCH, calls=calls,
        idx16=idx16, dcol=dcol, dis=dis, dis_cb=dis_cb, Kmax=Kmax,
        flags=[tuple(f) for f in flags], blk_of_ch=blk_of_ch,
        selfstop=selfstop,
    )


def make_inputs(x, W1, W2, W3, pp, KIN):
    import ml_dtypes
    bf = ml_dtypes.bfloat16
    F1, F2 = W1.shape[1], W2.shape[1]
    COUT = W3.shape[1]
    SHARD, PADN, NB = pp["SHARD"], pp["PADN"], pp["NB"]
    Kmax = pp["Kmax"]
    perm = pp["perm_of_node"]
    KK = KIN // 128

    xs = x.astype(np.float32) * pp["dis"][:, None]
    xs_p = np.zeros((PADN, KIN), np.float32)
    xs_p[perm] = xs
    # iota3[p, d, j] = d  (bf16, packed last dim for DVE 2x mode)
    iota3 = np.broadcast_to(
        np.arange(128, dtype=np.float32)[None, :, None],
        (128, 128, Kmax)).astype(bf)
    w1r = np.ascontiguousarray(
        W1.reshape(KK, 128, F1).transpose(1, 0, 2).reshape(128, KK * F1)
    ).astype(bf)
    in_maps = []
    for c in range(NCORES):
        sl = xs_p[c * SHARD:(c + 1) * SHARD, :]
        # xb[p, r, kk, cc] = xs[r*128+cc, kk*128+p]
        xb = np.ascontiguousarray(
            sl.reshape(NB, 128, KK, 128).transpose(3, 0, 2, 1)).astype(bf)
        in_maps.append({
            "xb": xb,
            "w1": w1r,
            "w2": np.ascontiguousarray(W2.astype(np.float32)).astype(bf),
            "w3": np.ascontiguousarray(W3.astype(np.float32)).astype(bf),
            "dcol": np.ascontiguousarray(pp["dcol"][c]).astype(bf),
            "idx": np.ascontiguousarray(pp["idx16"][c]),
            "iota3": np.ascontiguousarray(iota3),
            "dis": np.ascontiguousarray(pp["dis_cb"][c]),
            "dis2": np.ascontiguousarray(pp["dis_cb"][c] ** 2),
        })
    return in_maps


# ---------------- bass program builder ----------------


def build(params):
    NB = params["NB"]; NCH = params["NCH"]; calls = params["calls"]
    BUCKET = params["BUCKET"]; PADN = params["PADN"]
    KIN = params["KIN"]; F1 = params["F1"]; F2 = params["F2"]
    COUT = params["COUT"]
    Kmax = params["Kmax"]; flags = params["flags"]
    selfstop = params["selfstop"]
    REPEAT = params.get("repeat", 1)
    TIMING_LOOP = params.get("timing_loop", 0)
    MOCK_CC = params.get("mock_collectives", False)
    SHARD = NB * 128
    G = GRP
    NG = NB // G
    KK = KIN // 128
    WG = 14 if NB % 14 == 0 else (7 if NB % 7 == 0 else 1)

    calls_by_group = [[] for _ in range(NG)]
    for (g, q, ch0, runs) in calls:
        calls_by_group[g].append((q, ch0, runs))

    nc = bacc.Bacc(num_swdge_queues=1, dynamic_dma_scratch_size=65536)
    xb = nc.declare_dram_parameter("xb", [128, NB, KK, 128], BF, isOutput=False)
    w1 = nc.declare_dram_parameter("w1", [128, KK * F1], BF, isOutput=False)
    w2 = nc.declare_dram_parameter("w2", [F1, F2], BF, isOutput=False)
    w3 = nc.declare_dram_parameter("w3", [F2, COUT], BF, isOutput=False)
    dcol_in = nc.declare_dram_parameter("dcol", [128, NCH], BF, isOutput=False)
    idx_in = nc.declare_dram_parameter("idx", [128, NCH * 8], I16, isOutput=False)
    iota3_in = nc.declare_dram_parameter("iota3", [128, 128, Kmax], BF, isOutput=False)
    dis_in = nc.declare_dram_parameter("dis", [128, NB], FP, isOutput=False)
    dis2_in = nc.declare_dram_parameter("dis2", [128, NB], FP, isOutput=False)
    out_ext = nc.declare_dram_parameter("out", [SHARD, COUT], FP, isOutput=True)

    y_own = [nc.dram_tensor(f"y{l}_own", [SHARD, TW], BF) for l in (1, 2, 3)]
    y_full = [nc.dram_tensor(f"y{l}_full", [PADN, TW], BF, addr_space="Shared")
              for l in (1, 2, 3)]
    rg = [list(range(NCORES))]

    FDIM = [F1, F2, COUT]

    with TileContext(nc) as tc:
        with tc.tile_pool(name="const", bufs=1) as cpool, \
             tc.tile_pool(name="xt", bufs=3) as xpool, \
             tc.tile_pool(name="gt", bufs=1) as gtpool, \
             tc.tile_pool(name="ybuf", bufs=1) as ybpool, \
             tc.tile_pool(name="msg", bufs=4) as mpool, \
             tc.tile_pool(name="sgen", bufs=3) as spool, \
             tc.tile_pool(name="diag", bufs=3) as dpool, \
             tc.tile_pool(name="fin", bufs=4) as fpool, \
             tc.tile_pool(name="ps", bufs=1, space="PSUM") as pspool, \
             tc.tile_pool(name="pagg", bufs=G, space="PSUM") as papool:

            iota3 = cpool.tile([128, 128, Kmax], BF)
            nc.sync.dma_start(out=iota3[:], in_=iota3_in[:])
            dcol = cpool.tile([128, NCH], BF)
            nc.sync.dma_start(out=dcol[:], in_=dcol_in[:])
            idxsb = cpool.tile([128, NCH * 8], I16)
            nc.sync.dma_start(out=idxsb[:], in_=idx_in[:])
            dis = cpool.tile([128, NB], FP)
            nc.sync.dma_start(out=dis[:], in_=dis_in[:])
            dis2 = cpool.tile([128, NB], FP)
            nc.sync.dma_start(out=dis2[:], in_=dis2_in[:])
            w1sb = cpool.tile([128, KK * F1], BF)
            nc.sync.dma_start(out=w1sb[:], in_=w1[:])
            w2sb = cpool.tile([F1, F2], BF)
            nc.sync.dma_start(out=w2sb[:], in_=w2[:])
            w3sb = cpool.tile([F2, COUT], BF)
            nc.sync.dma_start(out=w3sb[:], in_=w3[:])
            ident = cpool.tile([128, 128], BF)
            make_identity(nc, ident[:])

            gT = gtpool.tile([128, SHARD], BF, tag="gT")
            ybuf = ybpool.tile([128, NB, TW], BF, tag="ybuf")

            def allgather(l):
                if MOCK_CC:
                    for s in range(NCORES):
                        nc.sync.dma_start(
                            out=y_full[l][s * SHARD:(s + 1) * SHARD, :],
                            in_=y_own[l][:])
                else:
                    nc.gpsimd.collective_compute(
                        "AllGather", mybir.AluOpType.bypass, replica_groups=rg,
                        ins=[y_own[l][:]], outs=[y_full[l][:]])

            def y_write(l, F):
                for gg in range(NB // WG):
                    nc.sync.dma_start(
                        out=y_own[l][gg * WG * 128:(gg + 1) * WG * 128, :F]
                            .rearrange("(c p) f -> p c f", p=128),
                        in_=ybuf[:, gg * WG:(gg + 1) * WG, :F])

            def build_table(l):
                """Write table l (0-based) into ybuf/y_own[l].

                l == 0: y1 = (dis*x) @ W1  (dis folded into xb host-side)
                l >  0: y_{l+1} = (dis^2 * gT') @ W_{l+1}, gT' = relu(agg)
                """
                F_out = FDIM[l]
                for r in range(NB):
                    ps = pspool.tile([128, F_out], FP, tag="psy")
                    if l == 0:
                        xt = xpool.tile([128, KK, 128], BF, tag="xt")
                        nc.sync.dma_start(out=xt[:], in_=xb[:, r, :, :])
                        for kk in range(KK):
                            nc.tensor.matmul(
                                ps[:], xt[:, kk, :],
                                w1sb[:, kk * F1:(kk + 1) * F1],
                                start=(kk == 0), stop=(kk == KK - 1))
                        nc.vector.tensor_copy(ybuf[:, r, :F_out], ps[:])
                    else:
                        wsb = w2sb if l == 1 else w3sb
                        F_in = FDIM[l - 1]
                        nc.tensor.matmul(
                            ps[:], gT[:F_in, r * 128:(r + 1) * 128], wsb[:],
                            start=True, stop=True)
                        nc.scalar.activation(
                            ybuf[:, r, :F_out], ps[:],
                            mybir.ActivationFunctionType.Copy,
                            scale=dis2[:, r:r + 1])
                y_write(l, F_out)

            def agg_phase(l, last, outbuf=None):
                """Aggregate table l over edges into gT (relu) or output."""
                F = FDIM[l]
                yf = y_full[l]
                for g in range(NG):
                    # one full 2KB PSUM bank per concurrently-open block
                    # accumulator: PSUM "zero regions" are bank-granular, so
                    # interleaved accumulation groups must not share a bank.
                    pas = []
                    for _bl in range(G):
                        pa_blk = papool.tile([128, 512], FP, tag="pa")
                        pas.append(pa_blk)
                    for bl in range(G):
                        b = g * G + bl
                        if not last:
                            nc.tensor.matmul(
                                pas[bl][:F, :128], ybuf[:, b, :F], ident[:],
                                start=True, stop=selfstop[b])
                        else:
                            nc.tensor.matmul(
                                pas[bl][:, :COUT], ident[:], ybuf[:, b, :COUT],
                                start=True, stop=selfstop[b])
                    for (q, ch0, runs) in calls_by_group[g]:
                        # split to <=KCAP chunks per gather: the ISA
                        # num_idxs field cannot encode arbitrarily large
                        # gathers (4095-idx limit observed empirically).
                        subruns = []
                        cur, cn = [], 0
                        for (b, k) in runs:
                            rem = k
                            while rem:
                                take = min(rem, KCAP - cn)
                                cur.append((b, take))
                                cn += take
                                rem -= take
                                if cn == KCAP:
                                    subruns.append(cur)
                                    cur, cn = [], 0
                        if cur:
                            subruns.append(cur)
                        c0 = ch0
                        for sub in subruns:
                            K = sum(k for _, k in sub)
                            msg = mpool.tile([128, KCAP, TW], BF, tag="msg")
                            nc.gpsimd.dma_gather(
                                msg[:, :K, :],
                                yf[q * BUCKET:(q + 1) * BUCKET, :],
                                idxsb[:, c0 * 8:(c0 + K) * 8],
                                K * 128, K * 128, TW, queue_num=0,
                            )
                            S = spool.tile([128, 128, KCAP], BF, tag="S")
                            nc.vector.tensor_tensor(
                                out=S[:, :, :K],
                                in0=dcol[:, c0:c0 + K].unsqueeze(1)
                                    .to_broadcast([128, 128, K]),
                                in1=iota3[:, :, :K],
                                op=mybir.AluOpType.is_equal,
                            )
                            j = 0
                            for (b, k) in sub:
                                bl = b - g * G
                                for _ in range(k):
                                    first, last_ch = flags[c0 + j]
                                    if not last:
                                        nc.tensor.matmul(
                                            pas[bl][:F, :128],
                                            msg[:, j, :F], S[:, :, j],
                                            start=first, stop=last_ch)
                                    else:
                                        nc.tensor.matmul(
                                            pas[bl][:, :COUT],
                                            S[:, :, j], msg[:, j, :COUT],
                                            start=first, stop=last_ch)
                                    j += 1
                            c0 += K
                    if not last:
                        for bl in range(G):
                            b = g * G + bl
                            nc.scalar.activation(
                                gT[:F, b * 128:(b + 1) * 128],
                                pas[bl][:F, :128],
                                mybir.ActivationFunctionType.Relu)
                    else:
                        for bl in range(G):
                            b = g * G + bl
                            z = fpool.tile([128, COUT], FP, tag="z")
                            nc.scalar.activation(
                                z[:], pas[bl][:, :COUT],
                                mybir.ActivationFunctionType.Copy,
                                scale=dis[:, b:b + 1])
                            nm = fpool.tile([128, 1], FP, tag="nm")
                            nc.vector.tensor_reduce(
                                nm[:], z[:], mybir.AxisListType.X,
                                mybir.AluOpType.max, negate=True)
                            e = fpool.tile([128, COUT], FP, tag="e")
                            nc.scalar.activation(
                                e[:], z[:], mybir.ActivationFunctionType.Exp,
                                bias=nm[:])
                            s = fpool.tile([128, 1], FP, tag="s")
                            nc.vector.tensor_reduce(
                                s[:], e[:], mybir.AxisListType.X,
                                mybir.AluOpType.add)
                            lg = fpool.tile([128, 1], FP, tag="lg")
                            nc.scalar.activation(
                                lg[:], s[:], mybir.ActivationFunctionType.Ln)
                            bb = fpool.tile([128, 1], FP, tag="bb")
                            nc.vector.tensor_tensor(
                                out=bb[:], in0=nm[:], in1=lg[:],
                                op=mybir.AluOpType.subtract)
                            nc.vector.tensor_scalar(
                                out=outbuf[:, b % WG, :], in0=z[:],
                                scalar1=bb[:], scalar2=None,
                                op0=mybir.AluOpType.add)
                            if b % WG == WG - 1:
                                g0 = b - (WG - 1)
                                nc.sync.dma_start(
                                    out=out_ext[g0 * 128:(b + 1) * 128, :]
                                        .rearrange("(c p) f -> p c f", p=128),
                                    in_=outbuf[:])
                                outbuf = fpool.tile([128, WG, COUT], FP,
                                                    tag="ob")

            def pipeline(with_ag):
                build_table(0)
                if with_ag: allgather(0)
                agg_phase(0, last=False)

                build_table(1)
                if with_ag: allgather(1)
                agg_phase(1, last=False)

                build_table(2)
                if with_ag: allgather(2)
                ob = fpool.tile([128, WG, COUT], FP, tag="ob")
                agg_phase(2, last=True, outbuf=ob)

            for _rep in range(REPEAT):
                pipeline(True)
            if TIMING_LOOP:
                with tc.For_i(0, TIMING_LOOP, 1) as _:
                    pipeline(False)

    nc.compile()
    return nc


_CACHE = {}


def _reference_numpy(x, edge_index, W1, b1, W2, b2, W3, b3):
    src = edge_index[0].astype(np.int64); dst = edge_index[1].astype(np.int64)
    N = x.shape[0]
    deg = np.bincount(dst, minlength=N) + 1.0
    dis = 1.0 / np.sqrt(deg)
    norm = (dis[src] * dis[dst]).astype(np.float32)

    def layer(xv, W, b):
        xw = xv @ W
        agg = np.zeros_like(xw)
        np.add.at(agg, dst, xw[src] * norm[:, None])
        agg += xw * (dis * dis)[:, None].astype(np.float32)
        return agg + b

    h1 = np.maximum(layer(x.astype(np.float32), W1, b1), 0)
    h2 = np.maximum(layer(h1, W2, b2), 0)
    z = layer(h2, W3, b3)
    m = z.max(axis=1, keepdims=True)
    return (z - m - np.log(np.exp(z - m).sum(axis=1, keepdims=True))).astype(np.float32)


def kernel(x, edge_index, W1, b1, W2, b2, W3, b3):
    x = np.asarray(x); edge_index = np.asarray(edge_index)
    W1 = np.asarray(W1, np.float32); W2 = np.asarray(W2, np.float32)
    W3 = np.asarray(W3, np.float32)
    b1 = np.asarray(b1, np.float32); b2 = np.asarray(b2, np.float32)
    b3 = np.asarray(b3, np.float32)
    if np.any(b1) or np.any(b2) or np.any(b3):
        # device kernel fuses the (spec-guaranteed zero) biases away
        return _reference_numpy(x, edge_index, W1, b1, W2, b2, W3, b3)

    KIN = x.shape[1]
    F1, F2 = W1.shape[1], W2.shape[1]
    COUT = W3.shape[1]
    pp = preprocess(edge_index, x.shape[0], NB_BLOCKS)
    in_maps = make_inputs(x, W1, W2, W3, pp, KIN)
    key = ("nc", pp["NCH"], tuple(pp["calls"]))
    if key not in _CACHE:
        params = dict(NB=NB_BLOCKS, NCH=pp["NCH"], calls=pp["calls"],
                      BUCKET=pp["BUCKET"], PADN=pp["PADN"], KIN=KIN,
                      F1=F1, F2=F2, COUT=COUT, Kmax=pp["Kmax"],
                      flags=pp["flags"], selfstop=pp["selfstop"])
        _CACHE[key] = build(params)
    nc = _CACHE[key]
    res = run_bass_kernel_spmd(nc, in_maps, list(range(NCORES)))
    full = np.concatenate([res.results[c]["out"] for c in range(NCORES)], axis=0)
    return np.ascontiguousarray(full[pp["perm_of_node"]]).astype(np.float32)


# revision 21
# speedup vs baseline: 2.4057x; 2.4057x over previous
"""Self-contained Trainium2 (Bass) kernel for the 3-layer GCN
nn_FeaturePropagationModule problem: 100K nodes, 1.6M edges,
dims 512->64->128->40, log_softmax output, 8 NeuronCores.

Strategy (sharding_hint: shard nodes + edges by destination, replicate
weights): nodes are permuted into 8 shards x 98 blocks x 128 dsts
(degree-balanced); per layer each core computes its shard's y table
(y = scaled h @ W), AllGathers the bf16 table, then aggregates its own
dst blocks via dma_gather of 256B source rows + one-hot scatter-add
matmuls accumulated in PSUM.

v2 (vs the first working version): gathers are grouped 7 dst-blocks at
a time (one gather per (block-group, src-bucket) instead of per
(block, bucket)) cutting SWDGE descriptor-generation fixed overhead
(994ns/instr) ~7x; the one-hot S is built in [edge, dst, chunk] layout
from all-bf16 packed operands so the DVE 2x_1p mode applies; L1/L2
aggregation runs transposed (paT = msg^T @ S) so the epilogue is a
single Relu straight into the next layer's lhsT table (no transpose /
copy), with the symmetric-norm scale folded into the next y-phase's
PSUM->SBUF copy (relu(d^2*agg) = d*relu(d*agg)); edge indices and
dst-column tables are SBUF-resident, loaded once; x is staged
block-major so L1 streams 1KB-contiguous tiles.
"""
import numpy as np

import concourse.bacc as bacc
import concourse.mybir as mybir
from concourse.bass_utils import run_bass_kernel_spmd
from concourse.masks import make_identity
from concourse.tile import TileContext

FP = mybir.dt.float32
BF = mybir.dt.bfloat16
I16 = mybir.dt.int16
TW = 128  # gather-table width (bf16 -> 256B rows)
KCAP = 8  # max chunks per dma_gather (1024-idx hard limit, probed)
NCORES = 8
N_NODES = 100000
NB_BLOCKS = 98
GRP = 7  # dst blocks per gather group


# ---------------- host-side preprocessing ----------------


def _balanced_assignment(src, dst, N, NB):
    """Two-stage node->position assignment minimizing chunk padding.

    Stage 1: nodes -> 4 core-pairs (= src buckets) by out-degree snake.
    Stage 2: per pair, round-based packing of nodes into the pair's
    2*NB (core, block) bins of 128, balancing each bin's 4-vector of
    per-bucket in-degrees (self-loops excluded: they never enter the
    gather path).
    """
    SHARD = NB * 128
    PADN = NCORES * SHARD

    outdeg = np.bincount(src, minlength=N) + 1.0
    order = np.argsort(-outdeg, kind="stable")
    pair_of_node = np.empty(N, dtype=np.int64)
    for i, n_ in enumerate(order):
        r, c = divmod(i, 4)
        pair_of_node[n_] = c if r % 2 == 0 else 3 - c

    indegq = np.zeros((N, 4), dtype=np.int32)
    np.add.at(indegq, (dst, pair_of_node[src]), 1)

    caps = np.full((NB, 4), 590.0)
    NBINP = 2 * NB
    cap_bins = np.vstack([caps, caps])  # [2*NB, 4]

    perm_of_node = np.full(N, -1, dtype=np.int64)
    node_of_perm = np.full(PADN, -1, dtype=np.int64)
    for p in range(4):
        nodes = np.where(pair_of_node == p)[0]
        w = indegq[nodes].astype(np.float64)
        o = np.argsort(-w.sum(axis=1), kind="stable")
        nodes, w = nodes[o], w[o]
        npair = len(nodes)
        loads = np.zeros((NBINP, 4))
        fill = np.zeros(NBINP, dtype=np.int64)
        assign = np.empty(npair, dtype=np.int64)
        pos = 0
        for r in range(128):
            take = min(NBINP, npair - pos)
            if take <= 0:
                break
            for i in range(pos, pos + take):
                newload = loads + w[i]
                relfill = (newload / cap_bins).max(axis=1)
                infeas = (fill != r) | (newload > cap_bins).any(axis=1)
                score = np.where(infeas, np.inf, relfill)
                bi = int(np.argmin(score))
                if not np.isfinite(score[bi]):
                    over = np.where(fill != r, np.inf,
                                    (newload - cap_bins).max(axis=1))
                    bi = int(np.argmin(over))
                assign[i] = bi
                loads[bi] += w[i]
                fill[bi] += 1
            pos += take
        cnt = np.zeros(NBINP, dtype=np.int64)
        for i, n_ in enumerate(nodes):
            bi = assign[i]
            core = 2 * p + bi // NB
            blk = bi % NB
            ppos = core * SHARD + blk * 128 + cnt[bi]
            cnt[bi] += 1
            perm_of_node[n_] = ppos
            node_of_perm[ppos] = n_
    return perm_of_node, node_of_perm


def preprocess(edge_index: np.ndarray, N: int, NB: int):
    G = GRP
    NG = NB // G
    assert NB % G == 0
    SHARD = NB * 128
    PADN = NCORES * SHARD
    BUCKET = PADN // 4
    assert BUCKET < 32768 and N <= PADN
    src = edge_index[0].astype(np.int64)
    dst = edge_index[1].astype(np.int64)

    deg = np.bincount(dst, minlength=N).astype(np.float64) + 1.0
    dis = (1.0 / np.sqrt(deg)).astype(np.float32)

    perm_of_node, node_of_perm = _balanced_assignment(src, dst, N, NB)

    # permuted edge list; self-loops are NOT gathered (their contribution
    # is added locally from ybuf via a diagonal-matmul PSUM seed)
    psrc = perm_of_node[src]
    pdst = perm_of_node[dst]

    core_of = pdst // SHARD
    blk_of = (pdst % SHARD) // 128
    dcol_of = pdst % 128
    bucket_of = psrc // BUCKET

    counts = np.zeros((NCORES, NB, 4), dtype=np.int64)
    np.add.at(counts, (core_of, blk_of, bucket_of), 1)
    kbq = np.ceil(counts / 128).astype(np.int64).max(axis=0)  # [NB, 4]

    # chunk layout: (group, bucket, block)-major
    calls = []  # (g, q, ch0, runs=((b, k), ...))
    ch = 0
    ch0_of_bq = np.full((NB, 4), -1, dtype=np.int64)
    for g in range(NG):
        for q in range(4):
            runs = []
            ch0 = ch
            for b in range(g * G, (g + 1) * G):
                k = int(kbq[b, q])
                if k == 0:
                    continue
                ch0_of_bq[b, q] = ch
                runs.append((b, k))
                ch += k
            if runs:
                calls.append((g, q, ch0, tuple(runs)))
    NCH = ch

    # per-chunk (is_first, is_last) for PSUM accumulation groups; the
    # group is STARTED by the per-block self-term matmul, so is_first is
    # always False. selfstop[b]: block has no gathered chunks at all.
    flags = [[False, False] for _ in range(NCH)]
    selfstop = [False] * NB
    blk_of_ch = np.zeros(NCH, dtype=np.int64)
    for b in range(NB):
        chs = []
        for q in range(4):
            k = int(kbq[b, q])
            if k:
                c0 = int(ch0_of_bq[b, q])
                chs.extend(range(c0, c0 + k))
        if not chs:
            selfstop[b] = True
            continue
        flags[chs[-1]][1] = True
        for c in chs:
            blk_of_ch[c] = b

    idx16 = np.zeros((NCORES, 128, NCH * 8), dtype=np.int16)
    dcol = np.full((NCORES, 128, NCH), 255, dtype=np.float32)
    for c in range(NCORES):
        m = core_of == c
        eb, eq, ed, es = blk_of[m], bucket_of[m], dcol_of[m], psrc[m]
        o = np.lexsort((es, eb, eq))
        eb, eq, ed, es = eb[o], eq[o], ed[o], es[o]
        # edges now sorted by (bucket, block, src); offsets per (q, b)
        key = eq * NB + eb
        orderqb = np.argsort(key, kind="stable")
        eb, eq, ed, es = eb[orderqb], eq[orderqb], ed[orderqb], es[orderqb]
        cnts = np.bincount(key[orderqb], minlength=4 * NB)
        starts = np.concatenate([[0], np.cumsum(cnts)])
        for q in range(4):
            for b in range(NB):
                cnt = int(cnts[q * NB + b])
                k = int(kbq[b, q])
                if k == 0:
                    assert cnt == 0
                    continue
                assert cnt <= k * 128
                p0 = int(starts[q * NB + b])
                loc = es[p0:p0 + cnt] - q * BUCKET
                dcs = ed[p0:p0 + cnt]
                c0 = int(ch0_of_bq[b, q])
                flat_idx = np.zeros(k * 128, dtype=np.int16)
                flat_idx[:cnt] = loc.astype(np.int16)
                flat_dc = np.full(k * 128, 255, dtype=np.float32)
                flat_dc[:cnt] = dcs.astype(np.float32)
                cols = flat_idx.reshape(k * 8, 16).T
                for gg in range(8):
                    idx16[c, gg * 16:(gg + 1) * 16,
                          c0 * 8:(c0 + k) * 8] = cols
                dcol[c, :, c0:c0 + k] = flat_dc.reshape(k, 128).T

    dis_pad = np.zeros(PADN, dtype=np.float32)
    real = node_of_perm >= 0
    dis_pad[real] = dis[node_of_perm[real]]
    dis_cb = dis_pad.reshape(NCORES, NB, 128).transpose(0, 2, 1).copy()

    Kmax = max(sum(k for _, k in runs) for (_, _, _, runs) in calls)

    return dict(
        perm_of_node=perm_of_node, node_of_perm=node_of_perm, PADN=PADN,
        SHARD=SHARD, NB=NB, BUCKET=BUCKET, NCH=NCH, calls=calls,
        idx16=idx16, dcol=dcol, dis=dis, dis_cb=dis_cb, Kmax=Kmax,
        flags=[tuple(f) for f in flags], blk_of_ch=blk_of_ch,
        selfstop=selfstop,
    )


def make_inputs(x, W1, W2, W3, pp, KIN):
    import ml_dtypes
    bf = ml_dtypes.bfloat16
    F1, F2 = W1.shape[1], W2.shape[1]
    COUT = W3.shape[1]
    SHARD, PADN, NB = pp["SHARD"], pp["PADN"], pp["NB"]
    Kmax = pp["Kmax"]
    perm = pp["perm_of_node"]
    KK = KIN // 128

    xs = x.astype(np.float32) * pp["dis"][:, None]
    xs_p = np.zeros((PADN, KIN), np.float32)
    xs_p[perm] = xs
    # iota3[p, d, j] = d  (bf16, packed last dim for DVE 2x mode)
    iota3 = np.broadcast_to(
        np.arange(128, dtype=np.float32)[None, :, None],
        (128, 128, Kmax)).astype(bf)
    w1r = np.ascontiguousarray(
        W1.reshape(KK, 128, F1).transpose(1, 0, 2).reshape(128, KK * F1)
    ).astype(bf)
    in_maps = []
    for c in range(NCORES):
        sl = xs_p[c * SHARD:(c + 1) * SHARD, :]
        # xb[p, r, kk, cc] = xs[r*128+cc, kk*128+p]
        xb = np.ascontiguousarray(
            sl.reshape(NB, 128, KK, 128).transpose(3, 0, 2, 1)).astype(bf)
        in_maps.append({
            "xb": xb,
            "w1": w1r,
            "w2": np.ascontiguousarray(W2.astype(np.float32)).astype(bf),
            "w3": np.ascontiguousarray(W3.astype(np.float32)).astype(bf),
            "dcol": np.ascontiguousarray(pp["dcol"][c]).astype(bf),
            "idx": np.ascontiguousarray(pp["idx16"][c]),
            "iota3": np.ascontiguousarray(iota3),
            "dis": np.ascontiguousarray(pp["dis_cb"][c]),
            "dis2": np.ascontiguousarray(pp["dis_cb"][c] ** 2),
        })
    return in_maps


# ---------------- bass program builder ----------------


def build(params):
    NB = params["NB"]; NCH = params["NCH"]; calls = params["calls"]
    BUCKET = params["BUCKET"]; PADN = params["PADN"]
    KIN = params["KIN"]; F1 = params["F1"]; F2 = params["F2"]
    COUT = params["COUT"]
    Kmax = params["Kmax"]; flags = params["flags"]
    selfstop = params["selfstop"]
    REPEAT = params.get("repeat", 1)
    TIMING_LOOP = params.get("timing_loop", 0)
    MOCK_CC = params.get("mock_collectives", False)
    SHARD = NB * 128
    G = GRP
    NG = NB // G
    KK = KIN // 128
    WG = 14 if NB % 14 == 0 else (7 if NB % 7 == 0 else 1)

    calls_by_group = [[] for _ in range(NG)]
    for (g, q, ch0, runs) in calls:
        calls_by_group[g].append((q, ch0, runs))

    nc = bacc.Bacc(num_swdge_queues=4, dynamic_dma_scratch_size=65536)
    xb = nc.declare_dram_parameter("xb", [128, NB, KK, 128], BF, isOutput=False)
    w1 = nc.declare_dram_parameter("w1", [128, KK * F1], BF, isOutput=False)
    w2 = nc.declare_dram_parameter("w2", [F1, F2], BF, isOutput=False)
    w3 = nc.declare_dram_parameter("w3", [F2, COUT], BF, isOutput=False)
    dcol_in = nc.declare_dram_parameter("dcol", [128, NCH], BF, isOutput=False)
    idx_in = nc.declare_dram_parameter("idx", [128, NCH * 8], I16, isOutput=False)
    iota3_in = nc.declare_dram_parameter("iota3", [128, 128, Kmax], BF, isOutput=False)
    dis_in = nc.declare_dram_parameter("dis", [128, NB], FP, isOutput=False)
    dis2_in = nc.declare_dram_parameter("dis2", [128, NB], FP, isOutput=False)
    out_ext = nc.declare_dram_parameter("out", [SHARD, COUT], FP, isOutput=True)

    y_own = [nc.dram_tensor(f"y{l}_own", [SHARD, TW], BF) for l in (1, 2, 3)]
    y_full = [nc.dram_tensor(f"y{l}_full", [PADN, TW], BF, addr_space="Shared")
              for l in (1, 2, 3)]
    rg = [list(range(NCORES))]

    FDIM = [F1, F2, COUT]
    gather_counter = [0]

    with TileContext(nc) as tc:
        with tc.tile_pool(name="const", bufs=1) as cpool, \
             tc.tile_pool(name="xt", bufs=3) as xpool, \
             tc.tile_pool(name="gt", bufs=1) as gtpool, \
             tc.tile_pool(name="ybuf", bufs=1) as ybpool, \
             tc.tile_pool(name="msg", bufs=4) as mpool, \
             tc.tile_pool(name="sgen", bufs=3) as spool, \
             tc.tile_pool(name="diag", bufs=3) as dpool, \
             tc.tile_pool(name="fin", bufs=4) as fpool, \
             tc.tile_pool(name="ps", bufs=1, space="PSUM") as pspool, \
             tc.tile_pool(name="pagg", bufs=G, space="PSUM") as papool:

            iota3 = cpool.tile([128, 128, Kmax], BF)
            nc.sync.dma_start(out=iota3[:], in_=iota3_in[:])
            dcol = cpool.tile([128, NCH], BF)
            nc.sync.dma_start(out=dcol[:], in_=dcol_in[:])
            idxsb = cpool.tile([128, NCH * 8], I16)
            nc.sync.dma_start(out=idxsb[:], in_=idx_in[:])
            dis = cpool.tile([128, NB], FP)
            nc.sync.dma_start(out=dis[:], in_=dis_in[:])
            dis2 = cpool.tile([128, NB], FP)
            nc.sync.dma_start(out=dis2[:], in_=dis2_in[:])
            w1sb = cpool.tile([128, KK * F1], BF)
            nc.sync.dma_start(out=w1sb[:], in_=w1[:])
            w2sb = cpool.tile([F1, F2], BF)
            nc.sync.dma_start(out=w2sb[:], in_=w2[:])
            w3sb = cpool.tile([F2, COUT], BF)
            nc.sync.dma_start(out=w3sb[:], in_=w3[:])
            ident = cpool.tile([128, 128], BF)
            make_identity(nc, ident[:])

            gT = gtpool.tile([128, SHARD], BF, tag="gT")
            ybuf = ybpool.tile([128, NB, TW], BF, tag="ybuf")

            def allgather(l):
                if MOCK_CC:
                    for s in range(NCORES):
                        nc.sync.dma_start(
                            out=y_full[l][s * SHARD:(s + 1) * SHARD, :],
                            in_=y_own[l][:])
                else:
                    nc.gpsimd.collective_compute(
                        "AllGather", mybir.AluOpType.bypass, replica_groups=rg,
                        ins=[y_own[l][:]], outs=[y_full[l][:]])

            def y_write(l, F):
                for gg in range(NB // WG):
                    nc.sync.dma_start(
                        out=y_own[l][gg * WG * 128:(gg + 1) * WG * 128, :F]
                            .rearrange("(c p) f -> p c f", p=128),
                        in_=ybuf[:, gg * WG:(gg + 1) * WG, :F])

            def build_table(l):
                """Write table l (0-based) into ybuf/y_own[l].

                l == 0: y1 = (dis*x) @ W1  (dis folded into xb host-side)
                l >  0: y_{l+1} = (dis^2 * gT') @ W_{l+1}, gT' = relu(agg)
                """
                F_out = FDIM[l]
                for r in range(NB):
                    ps = pspool.tile([128, F_out], FP, tag="psy")
                    if l == 0:
                        xt = xpool.tile([128, KK, 128], BF, tag="xt")
                        nc.sync.dma_start(out=xt[:], in_=xb[:, r, :, :])
                        for kk in range(KK):
                            nc.tensor.matmul(
                                ps[:], xt[:, kk, :],
                                w1sb[:, kk * F1:(kk + 1) * F1],
                                start=(kk == 0), stop=(kk == KK - 1))
                        nc.vector.tensor_copy(ybuf[:, r, :F_out], ps[:])
                    else:
                        wsb = w2sb if l == 1 else w3sb
                        F_in = FDIM[l - 1]
                        nc.tensor.matmul(
                            ps[:], gT[:F_in, r * 128:(r + 1) * 128], wsb[:],
                            start=True, stop=True)
                        nc.scalar.activation(
                            ybuf[:, r, :F_out], ps[:],
                            mybir.ActivationFunctionType.Copy,
                            scale=dis2[:, r:r + 1])
                y_write(l, F_out)

            def agg_phase(l, last, outbuf=None):
                """Aggregate table l over edges into gT (relu) or output."""
                F = FDIM[l]
                yf = y_full[l]
                for g in range(NG):
                    # one full 2KB PSUM bank per concurrently-open block
                    # accumulator: PSUM "zero regions" are bank-granular, so
                    # interleaved accumulation groups must not share a bank.
                    pas = []
                    for _bl in range(G):
                        pa_blk = papool.tile([128, 512], FP, tag="pa")
                        pas.append(pa_blk)
                    for bl in range(G):
                        b = g * G + bl
                        if not last:
                            nc.tensor.matmul(
                                pas[bl][:F, :128], ybuf[:, b, :F], ident[:],
                                start=True, stop=selfstop[b])
                        else:
                            nc.tensor.matmul(
                                pas[bl][:, :COUT], ident[:], ybuf[:, b, :COUT],
                                start=True, stop=selfstop[b])
                    for (q, ch0, runs) in calls_by_group[g]:
                        # split to <=KCAP chunks per gather: the ISA
                        # num_idxs field cannot encode arbitrarily large
                        # gathers (4095-idx limit observed empirically).
                        subruns = []
                        cur, cn = [], 0
                        for (b, k) in runs:
                            rem = k
                            while rem:
                                take = min(rem, KCAP - cn)
                                cur.append((b, take))
                                cn += take
                                rem -= take
                                if cn == KCAP:
                                    subruns.append(cur)
                                    cur, cn = [], 0
                        if cur:
                            subruns.append(cur)
                        c0 = ch0
                        for sub in subruns:
                            K = sum(k for _, k in sub)
                            msg = mpool.tile([128, KCAP, TW], BF, tag="msg")
                            # queue i%4 with the 8-lane DMASW rotation
                            # keeps each completion-sem lane on one queue
                            qn = gather_counter[0] % 4
                            gather_counter[0] += 1
                            nc.gpsimd.dma_gather(
                                msg[:, :K, :],
                                yf[q * BUCKET:(q + 1) * BUCKET, :],
                                idxsb[:, c0 * 8:(c0 + K) * 8],
                                K * 128, K * 128, TW, queue_num=qn,
                            )
                            S = spool.tile([128, 128, KCAP], BF, tag="S")
                            nc.vector.tensor_tensor(
                                out=S[:, :, :K],
                                in0=dcol[:, c0:c0 + K].unsqueeze(1)
                                    .to_broadcast([128, 128, K]),
                                in1=iota3[:, :, :K],
                                op=mybir.AluOpType.is_equal,
                            )
                            j = 0
                            for (b, k) in sub:
                                bl = b - g * G
                                for _ in range(k):
                                    first, last_ch = flags[c0 + j]
                                    if not last:
                                        nc.tensor.matmul(
                                            pas[bl][:F, :128],
                                            msg[:, j, :F], S[:, :, j],
                                            start=first, stop=last_ch)
                                    else:
                                        nc.tensor.matmul(
                                            pas[bl][:, :COUT],
                                            S[:, :, j], msg[:, j, :COUT],
                                            start=first, stop=last_ch)
                                    j += 1
                            c0 += K
                    if not last:
                        for bl in range(G):
                            b = g * G + bl
                            nc.scalar.activation(
                                gT[:F, b * 128:(b + 1) * 128],
                                pas[bl][:F, :128],
                                mybir.ActivationFunctionType.Relu)
                    else:
                        for bl in range(G):
                            b = g * G + bl
                            z = fpool.tile([128, COUT], FP, tag="z")
                            nc.scalar.activation(
                                z[:], pas[bl][:, :COUT],
                                mybir.ActivationFunctionType.Copy,
                                scale=dis[:, b:b + 1])
                            nm = fpool.tile([128, 1], FP, tag="nm")
                            nc.vector.tensor_reduce(
                                nm[:], z[:], mybir.AxisListType.X,
                                mybir.AluOpType.max, negate=True)
                            e = fpool.tile([128, COUT], FP, tag="e")
                            nc.scalar.activation(
                                e[:], z[:], mybir.ActivationFunctionType.Exp,
                                bias=nm[:])
                            s = fpool.tile([128, 1], FP, tag="s")
                            nc.vector.tensor_reduce(
                                s[:], e[:], mybir.AxisListType.X,
                                mybir.AluOpType.add)
                            lg = fpool.tile([128, 1], FP, tag="lg")
                            nc.scalar.activation(
                                lg[:], s[:], mybir.ActivationFunctionType.Ln)
                            bb = fpool.tile([128, 1], FP, tag="bb")
                            nc.vector.tensor_tensor(
                                out=bb[:], in0=nm[:], in1=lg[:],
                                op=mybir.AluOpType.subtract)
                            nc.vector.tensor_scalar(
                                out=outbuf[:, b % WG, :], in0=z[:],
                                scalar1=bb[:], scalar2=None,
                                op0=mybir.AluOpType.add)
                            if b % WG == WG - 1:
                                g0 = b - (WG - 1)
                                nc.sync.dma_start(
                                    out=out_ext[g0 * 128:(b + 1) * 128, :]
                                        .rearrange("(c p) f -> p c f", p=128),
                                    in_=outbuf[:])
                                outbuf = fpool.tile([128, WG, COUT], FP,
                                                    tag="ob")

            def pipeline(with_ag):
                build_table(0)
                if with_ag: allgather(0)
                agg_phase(0, last=False)

                build_table(1)
                if with_ag: allgather(1)
                agg_phase(1, last=False)

                build_table(2)
                if with_ag: allgather(2)
                ob = fpool.tile([128, WG, COUT], FP, tag="ob")
                agg_phase(2, last=True, outbuf=ob)

            for _rep in range(REPEAT):
                pipeline(not params.get("no_ag", False))
            if TIMING_LOOP:
                with tc.For_i(0, TIMING_LOOP, 1) as _:
                    pipeline(False)

    nc.compile()
    return nc


_CACHE = {}


def _reference_numpy(x, edge_index, W1, b1, W2, b2, W3, b3):
    src = edge_index[0].astype(np.int64); dst = edge_index[1].astype(np.int64)
    N = x.shape[0]
    deg = np.bincount(dst, minlength=N) + 1.0
    dis = 1.0 / np.sqrt(deg)
    norm = (dis[src] * dis[dst]).astype(np.float32)

    def layer(xv, W, b):
        xw = xv @ W
        agg = np.zeros_like(xw)
        np.add.at(agg, dst, xw[src] * norm[:, None])
        agg += xw * (dis * dis)[:, None].astype(np.float32)
        return agg + b

    h1 = np.maximum(layer(x.astype(np.float32), W1, b1), 0)
    h2 = np.maximum(layer(h1, W2, b2), 0)
    z = layer(h2, W3, b3)
    m = z.max(axis=1, keepdims=True)
    return (z - m - np.log(np.exp(z - m).sum(axis=1, keepdims=True))).astype(np.float32)


def kernel(x, edge_index, W1, b1, W2, b2, W3, b3):
    x = np.asarray(x); edge_index = np.asarray(edge_index)
    W1 = np.asarray(W1, np.float32); W2 = np.asarray(W2, np.float32)
    W3 = np.asarray(W3, np.float32)
    b1 = np.asarray(b1, np.float32); b2 = np.asarray(b2, np.float32)
    b3 = np.asarray(b3, np.float32)
    if np.any(b1) or np.any(b2) or np.any(b3):
        # device kernel fuses the (spec-guaranteed zero) biases away
        return _reference_numpy(x, edge_index, W1, b1, W2, b2, W3, b3)

    KIN = x.shape[1]
    F1, F2 = W1.shape[1], W2.shape[1]
    COUT = W3.shape[1]
    pp = preprocess(edge_index, x.shape[0], NB_BLOCKS)
    in_maps = make_inputs(x, W1, W2, W3, pp, KIN)
    key = ("nc", pp["NCH"], tuple(pp["calls"]))
    if key not in _CACHE:
        params = dict(NB=NB_BLOCKS, NCH=pp["NCH"], calls=pp["calls"],
                      BUCKET=pp["BUCKET"], PADN=pp["PADN"], KIN=KIN,
                      F1=F1, F2=F2, COUT=COUT, Kmax=pp["Kmax"],
                      flags=pp["flags"], selfstop=pp["selfstop"])
        _CACHE[key] = build(params)
    nc = _CACHE[key]
    res = run_bass_kernel_spmd(nc, in_maps, list(range(NCORES)))
    full = np.concatenate([res.results[c]["out"] for c in range(NCORES)], axis=0)
    return np.ascontiguousarray(full[pp["perm_of_node"]]).astype(np.float32)


# revision 22
# speedup vs baseline: 3.7367x; 1.5533x over previous
"""Self-contained Trainium2 (Bass) kernel for the 3-layer GCN
nn_FeaturePropagationModule problem: 100K nodes, 1.6M edges,
dims 512->64->128->40, log_softmax output, 8 NeuronCores.

Strategy (sharding_hint: shard nodes + edges by destination, replicate
weights): nodes are permuted into 8 shards x 98 blocks x 128 dsts
(degree-balanced); per layer each core computes its shard's y table
(y = scaled h @ W), AllGathers the bf16 table, then aggregates its own
dst blocks via dma_gather of 256B source rows + one-hot scatter-add
matmuls accumulated in PSUM.

v2 (vs the first working version): gathers are grouped 7 dst-blocks at
a time (one gather per (block-group, src-bucket) instead of per
(block, bucket)) cutting SWDGE descriptor-generation fixed overhead
(994ns/instr) ~7x; the one-hot S is built in [edge, dst, chunk] layout
from all-bf16 packed operands so the DVE 2x_1p mode applies; L1/L2
aggregation runs transposed (paT = msg^T @ S) so the epilogue is a
single Relu straight into the next layer's lhsT table (no transpose /
copy), with the symmetric-norm scale folded into the next y-phase's
PSUM->SBUF copy (relu(d^2*agg) = d*relu(d*agg)); edge indices and
dst-column tables are SBUF-resident, loaded once; x is staged
block-major so L1 streams 1KB-contiguous tiles.
"""
import numpy as np

import concourse.bacc as bacc
import concourse.mybir as mybir
from concourse.bass_utils import run_bass_kernel_spmd
from concourse.masks import make_identity
from concourse.tile import TileContext

FP = mybir.dt.float32
BF = mybir.dt.bfloat16
I16 = mybir.dt.int16
TW = 128  # gather-table width (bf16 -> 256B rows)
KCAP = 8  # max chunks per dma_gather (1024-idx hard limit, probed)
NCORES = 8
N_NODES = 100000
NB_BLOCKS = 98
GRP = 7  # dst blocks per gather group


# ---------------- host-side preprocessing ----------------


def _balanced_assignment(src, dst, N, NB):
    """Two-stage node->position assignment minimizing chunk padding.

    Stage 1: nodes -> 4 core-pairs (= src buckets) by out-degree snake.
    Stage 2: per pair, round-based packing of nodes into the pair's
    2*NB (core, block) bins of 128, balancing each bin's 4-vector of
    per-bucket in-degrees (self-loops excluded: they never enter the
    gather path).
    """
    SHARD = NB * 128
    PADN = NCORES * SHARD

    outdeg = np.bincount(src, minlength=N) + 1.0
    order = np.argsort(-outdeg, kind="stable")
    pair_of_node = np.empty(N, dtype=np.int64)
    for i, n_ in enumerate(order):
        r, c = divmod(i, 4)
        pair_of_node[n_] = c if r % 2 == 0 else 3 - c

    indegq = np.zeros((N, 4), dtype=np.int32)
    np.add.at(indegq, (dst, pair_of_node[src]), 1)

    caps = np.full((NB, 4), 590.0)
    NBINP = 2 * NB
    cap_bins = np.vstack([caps, caps])  # [2*NB, 4]

    perm_of_node = np.full(N, -1, dtype=np.int64)
    node_of_perm = np.full(PADN, -1, dtype=np.int64)
    for p in range(4):
        nodes = np.where(pair_of_node == p)[0]
        w = indegq[nodes].astype(np.float64)
        o = np.argsort(-w.sum(axis=1), kind="stable")
        nodes, w = nodes[o], w[o]
        npair = len(nodes)
        loads = np.zeros((NBINP, 4))
        fill = np.zeros(NBINP, dtype=np.int64)
        assign = np.empty(npair, dtype=np.int64)
        pos = 0
        for r in range(128):
            take = min(NBINP, npair - pos)
            if take <= 0:
                break
            for i in range(pos, pos + take):
                newload = loads + w[i]
                relfill = (newload / cap_bins).max(axis=1)
                infeas = (fill != r) | (newload > cap_bins).any(axis=1)
                score = np.where(infeas, np.inf, relfill)
                bi = int(np.argmin(score))
                if not np.isfinite(score[bi]):
                    over = np.where(fill != r, np.inf,
                                    (newload - cap_bins).max(axis=1))
                    bi = int(np.argmin(over))
                assign[i] = bi
                loads[bi] += w[i]
                fill[bi] += 1
            pos += take
        cnt = np.zeros(NBINP, dtype=np.int64)
        for i, n_ in enumerate(nodes):
            bi = assign[i]
            core = 2 * p + bi // NB
            blk = bi % NB
            ppos = core * SHARD + blk * 128 + cnt[bi]
            cnt[bi] += 1
            perm_of_node[n_] = ppos
            node_of_perm[ppos] = n_
    return perm_of_node, node_of_perm


def preprocess(edge_index: np.ndarray, N: int, NB: int):
    G = GRP
    NG = NB // G
    assert NB % G == 0
    SHARD = NB * 128
    PADN = NCORES * SHARD
    BUCKET = PADN // 4
    assert BUCKET < 32768 and N <= PADN
    src = edge_index[0].astype(np.int64)
    dst = edge_index[1].astype(np.int64)

    deg = np.bincount(dst, minlength=N).astype(np.float64) + 1.0
    dis = (1.0 / np.sqrt(deg)).astype(np.float32)

    perm_of_node, node_of_perm = _balanced_assignment(src, dst, N, NB)

    # permuted edge list; self-loops are NOT gathered (their contribution
    # is added locally from ybuf via a diagonal-matmul PSUM seed)
    psrc = perm_of_node[src]
    pdst = perm_of_node[dst]

    core_of = pdst // SHARD
    blk_of = (pdst % SHARD) // 128
    dcol_of = pdst % 128
    bucket_of = psrc // BUCKET

    counts = np.zeros((NCORES, NB, 4), dtype=np.int64)
    np.add.at(counts, (core_of, blk_of, bucket_of), 1)
    kbq = np.ceil(counts / 128).astype(np.int64).max(axis=0)  # [NB, 4]

    # chunk layout: (group, bucket, block)-major
    calls = []  # (g, q, ch0, runs=((b, k), ...))
    ch = 0
    ch0_of_bq = np.full((NB, 4), -1, dtype=np.int64)
    for g in range(NG):
        for q in range(4):
            runs = []
            ch0 = ch
            for b in range(g * G, (g + 1) * G):
                k = int(kbq[b, q])
                if k == 0:
                    continue
                ch0_of_bq[b, q] = ch
                runs.append((b, k))
                ch += k
            if runs:
                calls.append((g, q, ch0, tuple(runs)))
    NCH = ch

    # per-chunk (is_first, is_last) for PSUM accumulation groups; the
    # group is STARTED by the per-block self-term matmul, so is_first is
    # always False. selfstop[b]: block has no gathered chunks at all.
    flags = [[False, False] for _ in range(NCH)]
    selfstop = [False] * NB
    blk_of_ch = np.zeros(NCH, dtype=np.int64)
    for b in range(NB):
        chs = []
        for q in range(4):
            k = int(kbq[b, q])
            if k:
                c0 = int(ch0_of_bq[b, q])
                chs.extend(range(c0, c0 + k))
        if not chs:
            selfstop[b] = True
            continue
        flags[chs[-1]][1] = True
        for c in chs:
            blk_of_ch[c] = b

    idx16 = np.zeros((NCORES, 128, NCH * 8), dtype=np.int16)
    dcol = np.full((NCORES, 128, NCH), 255, dtype=np.float32)
    for c in range(NCORES):
        m = core_of == c
        eb, eq, ed, es = blk_of[m], bucket_of[m], dcol_of[m], psrc[m]
        o = np.lexsort((es, eb, eq))
        eb, eq, ed, es = eb[o], eq[o], ed[o], es[o]
        # edges now sorted by (bucket, block, src); offsets per (q, b)
        key = eq * NB + eb
        orderqb = np.argsort(key, kind="stable")
        eb, eq, ed, es = eb[orderqb], eq[orderqb], ed[orderqb], es[orderqb]
        cnts = np.bincount(key[orderqb], minlength=4 * NB)
        starts = np.concatenate([[0], np.cumsum(cnts)])
        for q in range(4):
            for b in range(NB):
                cnt = int(cnts[q * NB + b])
                k = int(kbq[b, q])
                if k == 0:
                    assert cnt == 0
                    continue
                assert cnt <= k * 128
                p0 = int(starts[q * NB + b])
                loc = es[p0:p0 + cnt] - q * BUCKET
                dcs = ed[p0:p0 + cnt]
                c0 = int(ch0_of_bq[b, q])
                flat_idx = np.zeros(k * 128, dtype=np.int16)
                flat_idx[:cnt] = loc.astype(np.int16)
                flat_dc = np.full(k * 128, 255, dtype=np.float32)
                flat_dc[:cnt] = dcs.astype(np.float32)
                cols = flat_idx.reshape(k * 8, 16).T
                for gg in range(8):
                    idx16[c, gg * 16:(gg + 1) * 16,
                          c0 * 8:(c0 + k) * 8] = cols
                dcol[c, :, c0:c0 + k] = flat_dc.reshape(k, 128).T

    dis_pad = np.zeros(PADN, dtype=np.float32)
    real = node_of_perm >= 0
    dis_pad[real] = dis[node_of_perm[real]]
    dis_cb = dis_pad.reshape(NCORES, NB, 128).transpose(0, 2, 1).copy()

    Kmax = max(sum(k for _, k in runs) for (_, _, _, runs) in calls)

    return dict(
        perm_of_node=perm_of_node, node_of_perm=node_of_perm, PADN=PADN,
        SHARD=SHARD, NB=NB, BUCKET=BUCKET, NCH=NCH, calls=calls,
        idx16=idx16, dcol=dcol, dis=dis, dis_cb=dis_cb, Kmax=Kmax,
        flags=[tuple(f) for f in flags], blk_of_ch=blk_of_ch,
        selfstop=selfstop,
    )


def make_inputs(x, W1, W2, W3, pp, KIN):
    import ml_dtypes
    bf = ml_dtypes.bfloat16
    F1, F2 = W1.shape[1], W2.shape[1]
    COUT = W3.shape[1]
    SHARD, PADN, NB = pp["SHARD"], pp["PADN"], pp["NB"]
    Kmax = pp["Kmax"]
    perm = pp["perm_of_node"]
    KK = KIN // 128

    xs = x.astype(np.float32) * pp["dis"][:, None]
    xs_p = np.zeros((PADN, KIN), np.float32)
    xs_p[perm] = xs
    # iota3[p, d, j] = d  (bf16, packed last dim for DVE 2x mode)
    iota3 = np.broadcast_to(
        np.arange(128, dtype=np.float32)[None, :, None],
        (128, 128, Kmax)).astype(bf)
    w1r = np.ascontiguousarray(
        W1.reshape(KK, 128, F1).transpose(1, 0, 2).reshape(128, KK * F1)
    ).astype(bf)
    in_maps = []
    for c in range(NCORES):
        sl = xs_p[c * SHARD:(c + 1) * SHARD, :]
        # xb[p, r, kk, cc] = xs[r*128+cc, kk*128+p]
        xb = np.ascontiguousarray(
            sl.reshape(NB, 128, KK, 128).transpose(3, 0, 2, 1)).astype(bf)
        in_maps.append({
            "xb": xb,
            "w1": w1r,
            "w2": np.ascontiguousarray(W2.astype(np.float32)).astype(bf),
            "w3": np.ascontiguousarray(W3.astype(np.float32)).astype(bf),
            "dcol": np.ascontiguousarray(pp["dcol"][c]).astype(bf),
            "idx": np.ascontiguousarray(pp["idx16"][c]),
            "iota3": np.ascontiguousarray(iota3),
            "dis": np.ascontiguousarray(pp["dis_cb"][c]),
            "dis2": np.ascontiguousarray(pp["dis_cb"][c] ** 2),
        })
    return in_maps


# ---------------- bass program builder ----------------


def build(params):
    NB = params["NB"]; NCH = params["NCH"]; calls = params["calls"]
    BUCKET = params["BUCKET"]; PADN = params["PADN"]
    KIN = params["KIN"]; F1 = params["F1"]; F2 = params["F2"]
    COUT = params["COUT"]
    Kmax = params["Kmax"]; flags = params["flags"]
    selfstop = params["selfstop"]
    REPEAT = params.get("repeat", 1)
    TIMING_LOOP = params.get("timing_loop", 0)
    MOCK_CC = params.get("mock_collectives", False)
    SHARD = NB * 128
    G = GRP
    NG = NB // G
    KK = KIN // 128
    WG = 14 if NB % 14 == 0 else (7 if NB % 7 == 0 else 1)

    calls_by_group = [[] for _ in range(NG)]
    for (g, q, ch0, runs) in calls:
        calls_by_group[g].append((q, ch0, runs))

    nc = bacc.Bacc(num_swdge_queues=4, dynamic_dma_scratch_size=65536)
    xb = nc.declare_dram_parameter("xb", [128, NB, KK, 128], BF, isOutput=False)
    w1 = nc.declare_dram_parameter("w1", [128, KK * F1], BF, isOutput=False)
    w2 = nc.declare_dram_parameter("w2", [F1, F2], BF, isOutput=False)
    w3 = nc.declare_dram_parameter("w3", [F2, COUT], BF, isOutput=False)
    dcol_in = nc.declare_dram_parameter("dcol", [128, NCH], BF, isOutput=False)
    idx_in = nc.declare_dram_parameter("idx", [128, NCH * 8], I16, isOutput=False)
    iota3_in = nc.declare_dram_parameter("iota3", [128, 128, Kmax], BF, isOutput=False)
    dis_in = nc.declare_dram_parameter("dis", [128, NB], FP, isOutput=False)
    dis2_in = nc.declare_dram_parameter("dis2", [128, NB], FP, isOutput=False)
    out_ext = nc.declare_dram_parameter("out", [SHARD, COUT], FP, isOutput=True)

    y_own = [nc.dram_tensor(f"y{l}_own", [SHARD, TW], BF) for l in (1, 2, 3)]
    y_full = [nc.dram_tensor(f"y{l}_full", [PADN, TW], BF, addr_space="Shared")
              for l in (1, 2, 3)]
    rg = [list(range(NCORES))]

    FDIM = [F1, F2, COUT]
    gather_counter = [0]

    with TileContext(nc) as tc:
        with tc.tile_pool(name="const", bufs=1) as cpool, \
             tc.tile_pool(name="xt", bufs=3) as xpool, \
             tc.tile_pool(name="gt", bufs=1) as gtpool, \
             tc.tile_pool(name="ybuf", bufs=1) as ybpool, \
             tc.tile_pool(name="msg", bufs=8) as mpool, \
             tc.tile_pool(name="sgen", bufs=4) as spool, \
             tc.tile_pool(name="diag", bufs=3) as dpool, \
             tc.tile_pool(name="fin", bufs=4) as fpool, \
             tc.tile_pool(name="ps", bufs=1, space="PSUM") as pspool, \
             tc.tile_pool(name="pagg", bufs=G, space="PSUM") as papool:

            iota3 = cpool.tile([128, 128, Kmax], BF)
            nc.sync.dma_start(out=iota3[:], in_=iota3_in[:])
            dcol = cpool.tile([128, NCH], BF)
            nc.sync.dma_start(out=dcol[:], in_=dcol_in[:])
            idxsb = cpool.tile([128, NCH * 8], I16)
            nc.sync.dma_start(out=idxsb[:], in_=idx_in[:])
            dis = cpool.tile([128, NB], FP)
            nc.sync.dma_start(out=dis[:], in_=dis_in[:])
            dis2 = cpool.tile([128, NB], FP)
            nc.sync.dma_start(out=dis2[:], in_=dis2_in[:])
            w1sb = cpool.tile([128, KK * F1], BF)
            nc.sync.dma_start(out=w1sb[:], in_=w1[:])
            w2sb = cpool.tile([F1, F2], BF)
            nc.sync.dma_start(out=w2sb[:], in_=w2[:])
            w3sb = cpool.tile([F2, COUT], BF)
            nc.sync.dma_start(out=w3sb[:], in_=w3[:])
            ident = cpool.tile([128, 128], BF)
            make_identity(nc, ident[:])

            gT = gtpool.tile([128, SHARD], BF, tag="gT")
            ybuf = ybpool.tile([128, NB, TW], BF, tag="ybuf")

            def allgather(l):
                if MOCK_CC:
                    for s in range(NCORES):
                        nc.sync.dma_start(
                            out=y_full[l][s * SHARD:(s + 1) * SHARD, :],
                            in_=y_own[l][:])
                else:
                    nc.gpsimd.collective_compute(
                        "AllGather", mybir.AluOpType.bypass, replica_groups=rg,
                        ins=[y_own[l][:]], outs=[y_full[l][:]])

            def y_write(l, F):
                for gg in range(NB // WG):
                    nc.sync.dma_start(
                        out=y_own[l][gg * WG * 128:(gg + 1) * WG * 128, :F]
                            .rearrange("(c p) f -> p c f", p=128),
                        in_=ybuf[:, gg * WG:(gg + 1) * WG, :F])

            def build_table(l):
                """Write table l (0-based) into ybuf/y_own[l].

                l == 0: y1 = (dis*x) @ W1  (dis folded into xb host-side)
                l >  0: y_{l+1} = (dis^2 * gT') @ W_{l+1}, gT' = relu(agg)
                """
                F_out = FDIM[l]
                for r in range(NB):
                    ps = pspool.tile([128, F_out], FP, tag="psy")
                    if l == 0:
                        xt = xpool.tile([128, KK, 128], BF, tag="xt")
                        nc.sync.dma_start(out=xt[:], in_=xb[:, r, :, :])
                        for kk in range(KK):
                            nc.tensor.matmul(
                                ps[:], xt[:, kk, :],
                                w1sb[:, kk * F1:(kk + 1) * F1],
                                start=(kk == 0), stop=(kk == KK - 1))
                        nc.vector.tensor_copy(ybuf[:, r, :F_out], ps[:])
                    else:
                        wsb = w2sb if l == 1 else w3sb
                        F_in = FDIM[l - 1]
                        nc.tensor.matmul(
                            ps[:], gT[:F_in, r * 128:(r + 1) * 128], wsb[:],
                            start=True, stop=True)
                        nc.scalar.activation(
                            ybuf[:, r, :F_out], ps[:],
                            mybir.ActivationFunctionType.Copy,
                            scale=dis2[:, r:r + 1])
                y_write(l, F_out)

            def agg_phase(l, last, outbuf=None):
                """Aggregate table l over edges into gT (relu) or output."""
                F = FDIM[l]
                yf = y_full[l]
                for g in range(NG):
                    # one full 2KB PSUM bank per concurrently-open block
                    # accumulator: PSUM "zero regions" are bank-granular, so
                    # interleaved accumulation groups must not share a bank.
                    pas = []
                    for _bl in range(G):
                        pa_blk = papool.tile([128, 512], FP, tag="pa")
                        pas.append(pa_blk)
                    for bl in range(G):
                        b = g * G + bl
                        if not last:
                            nc.tensor.matmul(
                                pas[bl][:F, :128], ybuf[:, b, :F], ident[:],
                                start=True, stop=selfstop[b])
                        else:
                            nc.tensor.matmul(
                                pas[bl][:, :COUT], ident[:], ybuf[:, b, :COUT],
                                start=True, stop=selfstop[b])
                    for (q, ch0, runs) in calls_by_group[g]:
                        # split to <=KCAP chunks per gather: the ISA
                        # num_idxs field cannot encode arbitrarily large
                        # gathers (4095-idx limit observed empirically).
                        subruns = []
                        cur, cn = [], 0
                        for (b, k) in runs:
                            rem = k
                            while rem:
                                take = min(rem, KCAP - cn)
                                cur.append((b, take))
                                cn += take
                                rem -= take
                                if cn == KCAP:
                                    subruns.append(cur)
                                    cur, cn = [], 0
                        if cur:
                            subruns.append(cur)
                        c0 = ch0
                        for sub in subruns:
                            K = sum(k for _, k in sub)
                            msg = mpool.tile([128, KCAP, TW], BF, tag="msg")
                            # queue i%4 with the 8-lane DMASW rotation
                            # keeps each completion-sem lane on one queue
                            qn = gather_counter[0] % 4
                            gather_counter[0] += 1
                            nc.gpsimd.dma_gather(
                                msg[:, :K, :],
                                yf[q * BUCKET:(q + 1) * BUCKET, :],
                                idxsb[:, c0 * 8:(c0 + K) * 8],
                                K * 128, K * 128, TW, queue_num=qn,
                            )
                            S = spool.tile([128, 128, KCAP], BF, tag="S")
                            nc.vector.tensor_tensor(
                                out=S[:, :, :K],
                                in0=dcol[:, c0:c0 + K].unsqueeze(1)
                                    .to_broadcast([128, 128, K]),
                                in1=iota3[:, :, :K],
                                op=mybir.AluOpType.is_equal,
                            )
                            j = 0
                            for (b, k) in sub:
                                bl = b - g * G
                                for _ in range(k):
                                    first, last_ch = flags[c0 + j]
                                    if not last:
                                        nc.tensor.matmul(
                                            pas[bl][:F, :128],
                                            msg[:, j, :F], S[:, :, j],
                                            start=first, stop=last_ch)
                                    else:
                                        nc.tensor.matmul(
                                            pas[bl][:, :COUT],
                                            S[:, :, j], msg[:, j, :COUT],
                                            start=first, stop=last_ch)
                                    j += 1
                            c0 += K
                    if not last:
                        for bl in range(G):
                            b = g * G + bl
                            nc.scalar.activation(
                                gT[:F, b * 128:(b + 1) * 128],
                                pas[bl][:F, :128],
                                mybir.ActivationFunctionType.Relu)
                    else:
                        for bl in range(G):
                            b = g * G + bl
                            z = fpool.tile([128, COUT], FP, tag="z")
                            nc.scalar.activation(
                                z[:], pas[bl][:, :COUT],
                                mybir.ActivationFunctionType.Copy,
                                scale=dis[:, b:b + 1])
                            nm = fpool.tile([128, 1], FP, tag="nm")
                            nc.vector.tensor_reduce(
                                nm[:], z[:], mybir.AxisListType.X,
                                mybir.AluOpType.max, negate=True)
                            e = fpool.tile([128, COUT], FP, tag="e")
                            nc.scalar.activation(
                                e[:], z[:], mybir.ActivationFunctionType.Exp,
                                bias=nm[:])
                            s = fpool.tile([128, 1], FP, tag="s")
                            nc.vector.tensor_reduce(
                                s[:], e[:], mybir.AxisListType.X,
                                mybir.AluOpType.add)
                            lg = fpool.tile([128, 1], FP, tag="lg")
                            nc.scalar.activation(
                                lg[:], s[:], mybir.ActivationFunctionType.Ln)
                            bb = fpool.tile([128, 1], FP, tag="bb")
                            nc.vector.tensor_tensor(
                                out=bb[:], in0=nm[:], in1=lg[:],
                                op=mybir.AluOpType.subtract)
                            nc.vector.tensor_scalar(
                                out=outbuf[:, b % WG, :], in0=z[:],
                                scalar1=bb[:], scalar2=None,
                                op0=mybir.AluOpType.add)
                            if b % WG == WG - 1:
                                g0 = b - (WG - 1)
                                nc.sync.dma_start(
                                    out=out_ext[g0 * 128:(b + 1) * 128, :]
                                        .rearrange("(c p) f -> p c f", p=128),
                                    in_=outbuf[:])
                                outbuf = fpool.tile([128, WG, COUT], FP,
                                                    tag="ob")

            def pipeline(with_ag):
                build_table(0)
                if with_ag: allgather(0)
                agg_phase(0, last=False)

                build_table(1)
                if with_ag: allgather(1)
                agg_phase(1, last=False)

                build_table(2)
                if with_ag: allgather(2)
                ob = fpool.tile([128, WG, COUT], FP, tag="ob")
                agg_phase(2, last=True, outbuf=ob)

            for _rep in range(REPEAT):
                pipeline(not params.get("no_ag", False))
            if TIMING_LOOP:
                with tc.For_i(0, TIMING_LOOP, 1) as _:
                    pipeline(False)

    nc.compile()
    return nc


_CACHE = {}


def _reference_numpy(x, edge_index, W1, b1, W2, b2, W3, b3):
    src = edge_index[0].astype(np.int64); dst = edge_index[1].astype(np.int64)
    N = x.shape[0]
    deg = np.bincount(dst, minlength=N) + 1.0
    dis = 1.0 / np.sqrt(deg)
    norm = (dis[src] * dis[dst]).astype(np.float32)

    def layer(xv, W, b):
        xw = xv @ W
        agg = np.zeros_like(xw)
        np.add.at(agg, dst, xw[src] * norm[:, None])
        agg += xw * (dis * dis)[:, None].astype(np.float32)
        return agg + b

    h1 = np.maximum(layer(x.astype(np.float32), W1, b1), 0)
    h2 = np.maximum(layer(h1, W2, b2), 0)
    z = layer(h2, W3, b3)
    m = z.max(axis=1, keepdims=True)
    return (z - m - np.log(np.exp(z - m).sum(axis=1, keepdims=True))).astype(np.float32)


def kernel(x, edge_index, W1, b1, W2, b2, W3, b3):
    x = np.asarray(x); edge_index = np.asarray(edge_index)
    W1 = np.asarray(W1, np.float32); W2 = np.asarray(W2, np.float32)
    W3 = np.asarray(W3, np.float32)
    b1 = np.asarray(b1, np.float32); b2 = np.asarray(b2, np.float32)
    b3 = np.asarray(b3, np.float32)
    if np.any(b1) or np.any(b2) or np.any(b3):
        # device kernel fuses the (spec-guaranteed zero) biases away
        return _reference_numpy(x, edge_index, W1, b1, W2, b2, W3, b3)

    KIN = x.shape[1]
    F1, F2 = W1.shape[1], W2.shape[1]
    COUT = W3.shape[1]
    pp = preprocess(edge_index, x.shape[0], NB_BLOCKS)
    in_maps = make_inputs(x, W1, W2, W3, pp, KIN)
    key = ("nc", pp["NCH"], tuple(pp["calls"]))
    if key not in _CACHE:
        params = dict(NB=NB_BLOCKS, NCH=pp["NCH"], calls=pp["calls"],
                      BUCKET=pp["BUCKET"], PADN=pp["PADN"], KIN=KIN,
                      F1=F1, F2=F2, COUT=COUT, Kmax=pp["Kmax"],
                      flags=pp["flags"], selfstop=pp["selfstop"])
        _CACHE[key] = build(params)
    nc = _CACHE[key]
    res = run_bass_kernel_spmd(nc, in_maps, list(range(NCORES)))
    full = np.concatenate([res.results[c]["out"] for c in range(NCORES)], axis=0)
    return np.ascontiguousarray(full[pp["perm_of_node"]]).astype(np.float32)


# revision 24
# speedup vs baseline: 3.7921x; 1.0148x over previous
"""Self-contained Trainium2 (Bass) kernel for the 3-layer GCN
nn_FeaturePropagationModule problem: 100K nodes, 1.6M edges,
dims 512->64->128->40, log_softmax output, 8 NeuronCores.

Strategy (sharding_hint: shard nodes + edges by destination, replicate
weights): nodes are permuted into 8 shards x 98 blocks x 128 dsts
(degree-balanced); per layer each core computes its shard's y table
(y = scaled h @ W), AllGathers the bf16 table, then aggregates its own
dst blocks via dma_gather of 256B source rows + one-hot scatter-add
matmuls accumulated in PSUM.

v2 (vs the first working version): gathers are grouped 7 dst-blocks at
a time (one gather per (block-group, src-bucket) instead of per
(block, bucket)) cutting SWDGE descriptor-generation fixed overhead
(994ns/instr) ~7x; the one-hot S is built in [edge, dst, chunk] layout
from all-bf16 packed operands so the DVE 2x_1p mode applies; L1/L2
aggregation runs transposed (paT = msg^T @ S) so the epilogue is a
single Relu straight into the next layer's lhsT table (no transpose /
copy), with the symmetric-norm scale folded into the next y-phase's
PSUM->SBUF copy (relu(d^2*agg) = d*relu(d*agg)); edge indices and
dst-column tables are SBUF-resident, loaded once; x is staged
block-major so L1 streams 1KB-contiguous tiles.
"""
import numpy as np

import concourse.bacc as bacc
import concourse.mybir as mybir
from concourse.bass_utils import run_bass_kernel_spmd
from concourse.masks import make_identity
from concourse.tile import TileContext

FP = mybir.dt.float32
BF = mybir.dt.bfloat16
I16 = mybir.dt.int16
TW = 128  # gather-table width (bf16 -> 256B rows)
KCAP = 8  # max chunks per dma_gather (1024-idx hard limit, probed)
NCORES = 8
N_NODES = 100000
NB_BLOCKS = 98
GRP = 7  # dst blocks per gather group


# ---------------- host-side preprocessing ----------------


def _balanced_assignment(src, dst, N, NB):
    """Two-stage node->position assignment minimizing chunk padding.

    Stage 1: nodes -> 4 core-pairs (= src buckets) by out-degree snake.
    Stage 2: per pair, round-based packing of nodes into the pair's
    2*NB (core, block) bins of 128, balancing each bin's 4-vector of
    per-bucket in-degrees (self-loops excluded: they never enter the
    gather path).
    """
    SHARD = NB * 128
    PADN = NCORES * SHARD

    outdeg = np.bincount(src, minlength=N) + 1.0
    order = np.argsort(-outdeg, kind="stable")
    pair_of_node = np.empty(N, dtype=np.int64)
    for i, n_ in enumerate(order):
        r, c = divmod(i, 4)
        pair_of_node[n_] = c if r % 2 == 0 else 3 - c

    indegq = np.zeros((N, 4), dtype=np.int32)
    np.add.at(indegq, (dst, pair_of_node[src]), 1)

    # mixed chunk targets: most cells aim at k=4 (<=512 rows), a spread
    # of k=5 cells absorbs the excess (mean cell load ~510)
    caps = np.full((NB, 4), 504.0)
    nbump = 120
    i = 0
    while nbump > 0:
        b = i % NB
        q = (i // NB + b) % 4
        if caps[b, q] < 600:
            caps[b, q] = 630.0
            nbump -= 1
        i += 1
    NBINP = 2 * NB
    cap_bins = np.vstack([caps, caps])  # [2*NB, 4]

    perm_of_node = np.full(N, -1, dtype=np.int64)
    node_of_perm = np.full(PADN, -1, dtype=np.int64)
    for p in range(4):
        nodes = np.where(pair_of_node == p)[0]
        w = indegq[nodes].astype(np.float64)
        o = np.argsort(-w.sum(axis=1), kind="stable")
        nodes, w = nodes[o], w[o]
        npair = len(nodes)
        loads = np.zeros((NBINP, 4))
        fill = np.zeros(NBINP, dtype=np.int64)
        assign = np.empty(npair, dtype=np.int64)
        pos = 0
        for r in range(128):
            take = min(NBINP, npair - pos)
            if take <= 0:
                break
            for i in range(pos, pos + take):
                newload = loads + w[i]
                relfill = (newload / cap_bins).max(axis=1)
                infeas = (fill != r) | (newload > cap_bins).any(axis=1)
                score = np.where(infeas, np.inf, relfill)
                bi = int(np.argmin(score))
                if not np.isfinite(score[bi]):
                    over = np.where(fill != r, np.inf,
                                    (newload - cap_bins).max(axis=1))
                    bi = int(np.argmin(over))
                assign[i] = bi
                loads[bi] += w[i]
                fill[bi] += 1
            pos += take
        cnt = np.zeros(NBINP, dtype=np.int64)
        for i, n_ in enumerate(nodes):
            bi = assign[i]
            core = 2 * p + bi // NB
            blk = bi % NB
            ppos = core * SHARD + blk * 128 + cnt[bi]
            cnt[bi] += 1
            perm_of_node[n_] = ppos
            node_of_perm[ppos] = n_
    return perm_of_node, node_of_perm


def preprocess(edge_index: np.ndarray, N: int, NB: int):
    G = GRP
    NG = NB // G
    assert NB % G == 0
    SHARD = NB * 128
    PADN = NCORES * SHARD
    BUCKET = PADN // 4
    assert BUCKET < 32768 and N <= PADN
    src = edge_index[0].astype(np.int64)
    dst = edge_index[1].astype(np.int64)

    deg = np.bincount(dst, minlength=N).astype(np.float64) + 1.0
    dis = (1.0 / np.sqrt(deg)).astype(np.float32)

    perm_of_node, node_of_perm = _balanced_assignment(src, dst, N, NB)

    # permuted edge list; self-loops are NOT gathered (their contribution
    # is added locally from ybuf via a diagonal-matmul PSUM seed)
    psrc = perm_of_node[src]
    pdst = perm_of_node[dst]

    core_of = pdst // SHARD
    blk_of = (pdst % SHARD) // 128
    dcol_of = pdst % 128
    bucket_of = psrc // BUCKET

    counts = np.zeros((NCORES, NB, 4), dtype=np.int64)
    np.add.at(counts, (core_of, blk_of, bucket_of), 1)
    kbq = np.ceil(counts / 128).astype(np.int64).max(axis=0)  # [NB, 4]

    # chunk layout: (group, bucket, block)-major
    calls = []  # (g, q, ch0, runs=((b, k), ...))
    ch = 0
    ch0_of_bq = np.full((NB, 4), -1, dtype=np.int64)
    for g in range(NG):
        for q in range(4):
            runs = []
            ch0 = ch
            for b in range(g * G, (g + 1) * G):
                k = int(kbq[b, q])
                if k == 0:
                    continue
                ch0_of_bq[b, q] = ch
                runs.append((b, k))
                ch += k
            if runs:
                calls.append((g, q, ch0, tuple(runs)))
    NCH = ch

    # per-chunk (is_first, is_last) for PSUM accumulation groups; the
    # group is STARTED by the per-block self-term matmul, so is_first is
    # always False. selfstop[b]: block has no gathered chunks at all.
    flags = [[False, False] for _ in range(NCH)]
    selfstop = [False] * NB
    blk_of_ch = np.zeros(NCH, dtype=np.int64)
    for b in range(NB):
        chs = []
        for q in range(4):
            k = int(kbq[b, q])
            if k:
                c0 = int(ch0_of_bq[b, q])
                chs.extend(range(c0, c0 + k))
        if not chs:
            selfstop[b] = True
            continue
        flags[chs[-1]][1] = True
        for c in chs:
            blk_of_ch[c] = b

    idx16 = np.zeros((NCORES, 128, NCH * 8), dtype=np.int16)
    dcol = np.full((NCORES, 128, NCH), 255, dtype=np.float32)
    for c in range(NCORES):
        m = core_of == c
        eb, eq, ed, es = blk_of[m], bucket_of[m], dcol_of[m], psrc[m]
        o = np.lexsort((es, eb, eq))
        eb, eq, ed, es = eb[o], eq[o], ed[o], es[o]
        # edges now sorted by (bucket, block, src); offsets per (q, b)
        key = eq * NB + eb
        orderqb = np.argsort(key, kind="stable")
        eb, eq, ed, es = eb[orderqb], eq[orderqb], ed[orderqb], es[orderqb]
        cnts = np.bincount(key[orderqb], minlength=4 * NB)
        starts = np.concatenate([[0], np.cumsum(cnts)])
        for q in range(4):
            for b in range(NB):
                cnt = int(cnts[q * NB + b])
                k = int(kbq[b, q])
                if k == 0:
                    assert cnt == 0
                    continue
                assert cnt <= k * 128
                p0 = int(starts[q * NB + b])
                loc = es[p0:p0 + cnt] - q * BUCKET
                dcs = ed[p0:p0 + cnt]
                c0 = int(ch0_of_bq[b, q])
                flat_idx = np.zeros(k * 128, dtype=np.int16)
                flat_idx[:cnt] = loc.astype(np.int16)
                flat_dc = np.full(k * 128, 255, dtype=np.float32)
                flat_dc[:cnt] = dcs.astype(np.float32)
                cols = flat_idx.reshape(k * 8, 16).T
                for gg in range(8):
                    idx16[c, gg * 16:(gg + 1) * 16,
                          c0 * 8:(c0 + k) * 8] = cols
                dcol[c, :, c0:c0 + k] = flat_dc.reshape(k, 128).T

    dis_pad = np.zeros(PADN, dtype=np.float32)
    real = node_of_perm >= 0
    dis_pad[real] = dis[node_of_perm[real]]
    dis_cb = dis_pad.reshape(NCORES, NB, 128).transpose(0, 2, 1).copy()

    Kmax = max(sum(k for _, k in runs) for (_, _, _, runs) in calls)

    return dict(
        perm_of_node=perm_of_node, node_of_perm=node_of_perm, PADN=PADN,
        SHARD=SHARD, NB=NB, BUCKET=BUCKET, NCH=NCH, calls=calls,
        idx16=idx16, dcol=dcol, dis=dis, dis_cb=dis_cb, Kmax=Kmax,
        flags=[tuple(f) for f in flags], blk_of_ch=blk_of_ch,
        selfstop=selfstop,
    )


def make_inputs(x, W1, W2, W3, pp, KIN):
    import ml_dtypes
    bf = ml_dtypes.bfloat16
    F1, F2 = W1.shape[1], W2.shape[1]
    COUT = W3.shape[1]
    SHARD, PADN, NB = pp["SHARD"], pp["PADN"], pp["NB"]
    Kmax = pp["Kmax"]
    perm = pp["perm_of_node"]
    KK = KIN // 128

    xs = x.astype(np.float32) * pp["dis"][:, None]
    xs_p = np.zeros((PADN, KIN), np.float32)
    xs_p[perm] = xs
    # iota3[p, d, j] = d  (bf16, packed last dim for DVE 2x mode)
    iota3 = np.broadcast_to(
        np.arange(128, dtype=np.float32)[None, :, None],
        (128, 128, Kmax)).astype(bf)
    w1r = np.ascontiguousarray(
        W1.reshape(KK, 128, F1).transpose(1, 0, 2).reshape(128, KK * F1)
    ).astype(bf)
    in_maps = []
    for c in range(NCORES):
        sl = xs_p[c * SHARD:(c + 1) * SHARD, :]
        # xb[p, r, kk, cc] = xs[r*128+cc, kk*128+p]
        xb = np.ascontiguousarray(
            sl.reshape(NB, 128, KK, 128).transpose(3, 0, 2, 1)).astype(bf)
        in_maps.append({
            "xb": xb,
            "w1": w1r,
            "w2": np.ascontiguousarray(W2.astype(np.float32)).astype(bf),
            "w3": np.ascontiguousarray(W3.astype(np.float32)).astype(bf),
            "dcol": np.ascontiguousarray(pp["dcol"][c]).astype(bf),
            "idx": np.ascontiguousarray(pp["idx16"][c]),
            "iota3": np.ascontiguousarray(iota3),
            "dis": np.ascontiguousarray(pp["dis_cb"][c]),
            "dis2": np.ascontiguousarray(pp["dis_cb"][c] ** 2),
        })
    return in_maps


# ---------------- bass program builder ----------------


def build(params):
    NB = params["NB"]; NCH = params["NCH"]; calls = params["calls"]
    BUCKET = params["BUCKET"]; PADN = params["PADN"]
    KIN = params["KIN"]; F1 = params["F1"]; F2 = params["F2"]
    COUT = params["COUT"]
    Kmax = params["Kmax"]; flags = params["flags"]
    selfstop = params["selfstop"]
    REPEAT = params.get("repeat", 1)
    TIMING_LOOP = params.get("timing_loop", 0)
    MOCK_CC = params.get("mock_collectives", False)
    SHARD = NB * 128
    G = GRP
    NG = NB // G
    KK = KIN // 128
    WG = 14 if NB % 14 == 0 else (7 if NB % 7 == 0 else 1)

    calls_by_group = [[] for _ in range(NG)]
    for (g, q, ch0, runs) in calls:
        calls_by_group[g].append((q, ch0, runs))

    nc = bacc.Bacc(num_swdge_queues=4, dynamic_dma_scratch_size=65536)
    xb = nc.declare_dram_parameter("xb", [128, NB, KK, 128], BF, isOutput=False)
    w1 = nc.declare_dram_parameter("w1", [128, KK * F1], BF, isOutput=False)
    w2 = nc.declare_dram_parameter("w2", [F1, F2], BF, isOutput=False)
    w3 = nc.declare_dram_parameter("w3", [F2, COUT], BF, isOutput=False)
    dcol_in = nc.declare_dram_parameter("dcol", [128, NCH], BF, isOutput=False)
    idx_in = nc.declare_dram_parameter("idx", [128, NCH * 8], I16, isOutput=False)
    iota3_in = nc.declare_dram_parameter("iota3", [128, 128, Kmax], BF, isOutput=False)
    dis_in = nc.declare_dram_parameter("dis", [128, NB], FP, isOutput=False)
    dis2_in = nc.declare_dram_parameter("dis2", [128, NB], FP, isOutput=False)
    out_ext = nc.declare_dram_parameter("out", [SHARD, COUT], FP, isOutput=True)

    y_own = [nc.dram_tensor(f"y{l}_own", [SHARD, TW], BF) for l in (1, 2, 3)]
    y_full = [nc.dram_tensor(f"y{l}_full", [PADN, TW], BF, addr_space="Shared")
              for l in (1, 2, 3)]
    rg = [list(range(NCORES))]

    FDIM = [F1, F2, COUT]
    gather_counter = [0]

    with TileContext(nc) as tc:
        with tc.tile_pool(name="const", bufs=1) as cpool, \
             tc.tile_pool(name="xt", bufs=3) as xpool, \
             tc.tile_pool(name="gt", bufs=1) as gtpool, \
             tc.tile_pool(name="ybuf", bufs=1) as ybpool, \
             tc.tile_pool(name="msg", bufs=10) as mpool, \
             tc.tile_pool(name="sgen", bufs=5) as spool, \
             tc.tile_pool(name="diag", bufs=3) as dpool, \
             tc.tile_pool(name="fin", bufs=4) as fpool, \
             tc.tile_pool(name="ps", bufs=1, space="PSUM") as pspool, \
             tc.tile_pool(name="pagg", bufs=G, space="PSUM") as papool:

            iota3 = cpool.tile([128, 128, Kmax], BF)
            nc.sync.dma_start(out=iota3[:], in_=iota3_in[:])
            dcol = cpool.tile([128, NCH], BF)
            nc.sync.dma_start(out=dcol[:], in_=dcol_in[:])
            idxsb = cpool.tile([128, NCH * 8], I16)
            nc.sync.dma_start(out=idxsb[:], in_=idx_in[:])
            dis = cpool.tile([128, NB], FP)
            nc.sync.dma_start(out=dis[:], in_=dis_in[:])
            dis2 = cpool.tile([128, NB], FP)
            nc.sync.dma_start(out=dis2[:], in_=dis2_in[:])
            w1sb = cpool.tile([128, KK * F1], BF)
            nc.sync.dma_start(out=w1sb[:], in_=w1[:])
            w2sb = cpool.tile([F1, F2], BF)
            nc.sync.dma_start(out=w2sb[:], in_=w2[:])
            w3sb = cpool.tile([F2, COUT], BF)
            nc.sync.dma_start(out=w3sb[:], in_=w3[:])
            ident = cpool.tile([128, 128], BF)
            make_identity(nc, ident[:])

            gT = gtpool.tile([128, SHARD], BF, tag="gT")
            ybuf = ybpool.tile([128, NB, TW], BF, tag="ybuf")

            def allgather(l):
                if MOCK_CC:
                    for s in range(NCORES):
                        nc.sync.dma_start(
                            out=y_full[l][s * SHARD:(s + 1) * SHARD, :],
                            in_=y_own[l][:])
                else:
                    nc.gpsimd.collective_compute(
                        "AllGather", mybir.AluOpType.bypass, replica_groups=rg,
                        ins=[y_own[l][:]], outs=[y_full[l][:]])

            def y_write(l, F):
                for gg in range(NB // WG):
                    nc.sync.dma_start(
                        out=y_own[l][gg * WG * 128:(gg + 1) * WG * 128, :F]
                            .rearrange("(c p) f -> p c f", p=128),
                        in_=ybuf[:, gg * WG:(gg + 1) * WG, :F])

            def build_table(l):
                """Write table l (0-based) into ybuf/y_own[l].

                l == 0: y1 = (dis*x) @ W1  (dis folded into xb host-side)
                l >  0: y_{l+1} = (dis^2 * gT') @ W_{l+1}, gT' = relu(agg)
                """
                F_out = FDIM[l]
                for r in range(NB):
                    ps = pspool.tile([128, F_out], FP, tag="psy")
                    if l == 0:
                        xt = xpool.tile([128, KK, 128], BF, tag="xt")
                        nc.sync.dma_start(out=xt[:], in_=xb[:, r, :, :])
                        for kk in range(KK):
                            nc.tensor.matmul(
                                ps[:], xt[:, kk, :],
                                w1sb[:, kk * F1:(kk + 1) * F1],
                                start=(kk == 0), stop=(kk == KK - 1))
                        nc.vector.tensor_copy(ybuf[:, r, :F_out], ps[:])
                    else:
                        wsb = w2sb if l == 1 else w3sb
                        F_in = FDIM[l - 1]
                        nc.tensor.matmul(
                            ps[:], gT[:F_in, r * 128:(r + 1) * 128], wsb[:],
                            start=True, stop=True)
                        nc.scalar.activation(
                            ybuf[:, r, :F_out], ps[:],
                            mybir.ActivationFunctionType.Copy,
                            scale=dis2[:, r:r + 1])
                y_write(l, F_out)

            def agg_phase(l, last, outbuf=None):
                """Aggregate table l over edges into gT (relu) or output."""
                F = FDIM[l]
                yf = y_full[l]
                for g in range(NG):
                    # one full 2KB PSUM bank per concurrently-open block
                    # accumulator: PSUM "zero regions" are bank-granular, so
                    # interleaved accumulation groups must not share a bank.
                    pas = []
                    for _bl in range(G):
                        pa_blk = papool.tile([128, 512], FP, tag="pa")
                        pas.append(pa_blk)
                    for bl in range(G):
                        b = g * G + bl
                        if not last:
                            nc.tensor.matmul(
                                pas[bl][:F, :128], ybuf[:, b, :F], ident[:],
                                start=True, stop=selfstop[b])
                        else:
                            nc.tensor.matmul(
                                pas[bl][:, :COUT], ident[:], ybuf[:, b, :COUT],
                                start=True, stop=selfstop[b])
                    for (q, ch0, runs) in calls_by_group[g]:
                        # split to <=KCAP chunks per gather: the ISA
                        # num_idxs field cannot encode arbitrarily large
                        # gathers (4095-idx limit observed empirically).
                        subruns = []
                        cur, cn = [], 0
                        for (b, k) in runs:
                            rem = k
                            while rem:
                                take = min(rem, KCAP - cn)
                                cur.append((b, take))
                                cn += take
                                rem -= take
                                if cn == KCAP:
                                    subruns.append(cur)
                                    cur, cn = [], 0
                        if cur:
                            subruns.append(cur)
                        c0 = ch0
                        for sub in subruns:
                            K = sum(k for _, k in sub)
                            msg = mpool.tile([128, KCAP, TW], BF, tag="msg")
                            # queue i%4 with the 8-lane DMASW rotation
                            # keeps each completion-sem lane on one queue
                            qn = gather_counter[0] % 4
                            gather_counter[0] += 1
                            nc.gpsimd.dma_gather(
                                msg[:, :K, :],
                                yf[q * BUCKET:(q + 1) * BUCKET, :],
                                idxsb[:, c0 * 8:(c0 + K) * 8],
                                K * 128, K * 128, TW, queue_num=qn,
                            )
                            S = spool.tile([128, 128, KCAP], BF, tag="S")
                            nc.vector.tensor_tensor(
                                out=S[:, :, :K],
                                in0=dcol[:, c0:c0 + K].unsqueeze(1)
                                    .to_broadcast([128, 128, K]),
                                in1=iota3[:, :, :K],
                                op=mybir.AluOpType.is_equal,
                            )
                            j = 0
                            for (b, k) in sub:
                                bl = b - g * G
                                for _ in range(k):
                                    first, last_ch = flags[c0 + j]
                                    if not last:
                                        nc.tensor.matmul(
                                            pas[bl][:F, :128],
                                            msg[:, j, :F], S[:, :, j],
                                            start=first, stop=last_ch)
                                    else:
                                        nc.tensor.matmul(
                                            pas[bl][:, :COUT],
                                            S[:, :, j], msg[:, j, :COUT],
                                            start=first, stop=last_ch)
                                    j += 1
                            c0 += K
                    if not last:
                        for bl in range(G):
                            b = g * G + bl
                            nc.scalar.activation(
                                gT[:F, b * 128:(b + 1) * 128],
                                pas[bl][:F, :128],
                                mybir.ActivationFunctionType.Relu)
                    else:
                        for bl in range(G):
                            b = g * G + bl
                            z = fpool.tile([128, COUT], FP, tag="z")
                            nc.scalar.activation(
                                z[:], pas[bl][:, :COUT],
                                mybir.ActivationFunctionType.Copy,
                                scale=dis[:, b:b + 1])
                            nm = fpool.tile([128, 1], FP, tag="nm")
                            nc.vector.tensor_reduce(
                                nm[:], z[:], mybir.AxisListType.X,
                                mybir.AluOpType.max, negate=True)
                            e = fpool.tile([128, COUT], FP, tag="e")
                            nc.scalar.activation(
                                e[:], z[:], mybir.ActivationFunctionType.Exp,
                                bias=nm[:])
                            s = fpool.tile([128, 1], FP, tag="s")
                            nc.vector.tensor_reduce(
                                s[:], e[:], mybir.AxisListType.X,
                                mybir.AluOpType.add)
                            lg = fpool.tile([128, 1], FP, tag="lg")
                            nc.scalar.activation(
                                lg[:], s[:], mybir.ActivationFunctionType.Ln)
                            bb = fpool.tile([128, 1], FP, tag="bb")
                            nc.vector.tensor_tensor(
                                out=bb[:], in0=nm[:], in1=lg[:],
                                op=mybir.AluOpType.subtract)
                            nc.vector.tensor_scalar(
                                out=outbuf[:, b % WG, :], in0=z[:],
                                scalar1=bb[:], scalar2=None,
                                op0=mybir.AluOpType.add)
                            if b % WG == WG - 1:
                                g0 = b - (WG - 1)
                                nc.sync.dma_start(
                                    out=out_ext[g0 * 128:(b + 1) * 128, :]
                                        .rearrange("(c p) f -> p c f", p=128),
                                    in_=outbuf[:])
                                outbuf = fpool.tile([128, WG, COUT], FP,
                                                    tag="ob")

            def pipeline(with_ag):
                build_table(0)
                if with_ag: allgather(0)
                agg_phase(0, last=False)

                build_table(1)
                if with_ag: allgather(1)
                agg_phase(1, last=False)

                build_table(2)
                if with_ag: allgather(2)
                ob = fpool.tile([128, WG, COUT], FP, tag="ob")
                agg_phase(2, last=True, outbuf=ob)

            for _rep in range(REPEAT):
                pipeline(not params.get("no_ag", False))
            if TIMING_LOOP:
                with tc.For_i(0, TIMING_LOOP, 1) as _:
                    pipeline(False)

    nc.compile()
    return nc


_CACHE = {}


def _reference_numpy(x, edge_index, W1, b1, W2, b2, W3, b3):
    src = edge_index[0].astype(np.int64); dst = edge_index[1].astype(np.int64)
    N = x.shape[0]
    deg = np.bincount(dst, minlength=N) + 1.0
    dis = 1.0 / np.sqrt(deg)
    norm = (dis[src] * dis[dst]).astype(np.float32)

    def layer(xv, W, b):
        xw = xv @ W
        agg = np.zeros_like(xw)
        np.add.at(agg, dst, xw[src] * norm[:, None])
        agg += xw * (dis * dis)[:, None].astype(np.float32)
        return agg + b

    h1 = np.maximum(layer(x.astype(np.float32), W1, b1), 0)
    h2 = np.maximum(layer(h1, W2, b2), 0)
    z = layer(h2, W3, b3)
    m = z.max(axis=1, keepdims=True)
    return (z - m - np.log(np.exp(z - m).sum(axis=1, keepdims=True))).astype(np.float32)


def kernel(x, edge_index, W1, b1, W2, b2, W3, b3):
    x = np.asarray(x); edge_index = np.asarray(edge_index)
    W1 = np.asarray(W1, np.float32); W2 = np.asarray(W2, np.float32)
    W3 = np.asarray(W3, np.float32)
    b1 = np.asarray(b1, np.float32); b2 = np.asarray(b2, np.float32)
    b3 = np.asarray(b3, np.float32)
    if np.any(b1) or np.any(b2) or np.any(b3):
        # device kernel fuses the (spec-guaranteed zero) biases away
        return _reference_numpy(x, edge_index, W1, b1, W2, b2, W3, b3)

    KIN = x.shape[1]
    F1, F2 = W1.shape[1], W2.shape[1]
    COUT = W3.shape[1]
    pp = preprocess(edge_index, x.shape[0], NB_BLOCKS)
    in_maps = make_inputs(x, W1, W2, W3, pp, KIN)
    key = ("nc", pp["NCH"], tuple(pp["calls"]))
    if key not in _CACHE:
        params = dict(NB=NB_BLOCKS, NCH=pp["NCH"], calls=pp["calls"],
                      BUCKET=pp["BUCKET"], PADN=pp["PADN"], KIN=KIN,
                      F1=F1, F2=F2, COUT=COUT, Kmax=pp["Kmax"],
                      flags=pp["flags"], selfstop=pp["selfstop"])
        _CACHE[key] = build(params)
    nc = _CACHE[key]
    res = run_bass_kernel_spmd(nc, in_maps, list(range(NCORES)))
    full = np.concatenate([res.results[c]["out"] for c in range(NCORES)], axis=0)
    return np.ascontiguousarray(full[pp["perm_of_node"]]).astype(np.float32)
